# revision 26
# baseline (speedup 1.0000x reference)
"""BitTransformerEncoderLayer on 8 TRN2 NeuronCores.

Strategy: pure data parallelism over batch (B=8 == n_cores); no collectives.
v2: fp8e4 DoubleRow matmuls (2x PE rate) for in_proj / out_proj / attn@v /
softmax-denominator; scores and the BitLinear FFN matmuls stay bf16 (the FFN
runs exact integer arithmetic in bf16 — fp8 would round ints > 16 and blow the
error budget). h2 spills to DRAM as bf16. Softmax denominators via DVE tree +
one ones-matmul; reciprocal_approx_fast for 1/denom.

kernel(**inputs) takes the FULL unsharded inputs and returns the FULL output.
"""

import numpy as np

P = 128
EPS = 1e-8
MAGIC = 12582912.0  # 1.5 * 2**23: fp32 add/sub rounds to nearest-even integer
NCORES = 8
EXP_SHIFT = 2.0     # exp(scores - 2): max exp ~125 < 240 (TRN e4m3 max)
WSCALE = 64.0       # fp8 weight pre-scale for in_proj/out_proj
VSCALE = 16.0       # fp8 v pre-scale
OSCALE = 32.0       # fp8 o (attn out) pre-scale

# Problem dims (hardcoded per spec)
B_FULL, S_FULL, D_FULL, H_FULL, DFF_FULL = 8, 1024, 2048, 16, 8192

_CACHE = {}


# ---------------------------------------------------------------- host prep

def _quant_w(w):
    scale = np.maximum(np.mean(np.abs(w), dtype=np.float32), np.float32(1e-5))
    q = np.clip(np.round(w / scale), -1.0, 1.0).astype(np.float32)
    return q, float(scale)


def _lhsT_blocks(w):
    """w [M, K] -> [M/P, P(k), K/P, P(m)]; [mo, :, ko, :] = w-block(mo, ko).T"""
    M, K = w.shape
    t = w.reshape(M // P, P, K // P, P)  # [mo, pm, ko, pk]
    return np.ascontiguousarray(t.transpose(0, 3, 2, 1))


def _rhs_chunks(w, nch):
    """w [N, K] -> [N/nch, K/P, P, nch]; [no, ko, p, j] = w[no*nch+j, ko*P+p]"""
    N, K = w.shape
    t = w.reshape(N // nch, nch, K // P, P)  # [no, j, ko, p]
    return np.ascontiguousarray(t.transpose(0, 2, 3, 1))


def _rhs_pair_chunks(w, nch):
    """w [N, K] -> [N/nch, K/2P, P, 2, nch]; [no, k2, p, i, j] = w[no*nch+j, (2*k2+i)*P+p]"""
    N, K = w.shape
    t = w.reshape(N // nch, nch, K // (2 * P), 2, P)  # [no, j, k2, i, p]
    return np.ascontiguousarray(t.transpose(0, 2, 4, 3, 1))


def _per_part(v):
    """[M] -> [P, M/P]; out[p, mo] = v[mo*P + p]"""
    return np.ascontiguousarray(v.reshape(-1, P).T)


def _bcast_row(v):
    return np.ascontiguousarray(np.broadcast_to(v[None, :], (P, v.shape[0])))


def _prep_arrays(inputs, S, D, H, DFF):
    import ml_dtypes

    bf16 = ml_dtypes.bfloat16
    f8 = ml_dtypes.float8_e4m3fn
    f32 = np.float32
    g = lambda k: np.asarray(inputs[k], dtype=f32)

    w1q, ws1 = _quant_w(g("ff1_w"))   # [DFF, D]
    w2q, ws2 = _quant_w(g("ff2_w"))   # [D, DFF]
    ncd = min(512, D)

    bias_in = _per_part(g("in_proj_b")).astype(f32)  # [P, 3D/P]
    bias_in[:, 2 * (D // P):] *= VSCALE  # v eviction is pre-scaled by VSCALE

    arrays = {
        "w_in_blk": _lhsT_blocks(g("in_proj_w") * WSCALE).astype(f8),  # [3D/P, P, D/P, P]
        "wo_pair": _rhs_pair_chunks(g("out_proj_w") * WSCALE, ncd).astype(f8),
        "w1_blk": _lhsT_blocks(w1q).astype(bf16),                # [DFF/P, P, D/P, P]
        "w2_chunk": _rhs_chunks(w2q, ncd).astype(bf16),          # [D/ncd, DFF/P, P, ncd]
        "bias_in": bias_in,                                      # [P, 3D/P]
        "b1_t": _per_part(g("ff1_b")).astype(f32),               # [P, DFF/P]
        "alpha_t": _per_part(g("alpha")).astype(f32),
        "ab1_t": _per_part((g("alpha") * g("ff1_b")).astype(f32)),
        "gamma_t": _per_part((1.0 / (g("beta") + np.float32(1e-9))).astype(f32)),
        "n1w_bc": _bcast_row(g("norm1_w")).astype(f32),          # [P, D]
        "n2w_bc": _bcast_row(g("norm2_w")).astype(f32),
        "b2_bc": _bcast_row(g("ff2_b")).astype(f32),
    }
    return arrays, ws1, ws2


# ---------------------------------------------------------------- device program

def build_program(nc, *, S, D, H, DFF, ws1, ws2):
    import concourse.mybir as mybir
    import concourse.tile as tile
    from concourse.bass import ts
    from concourse.masks import make_identity

    dt = mybir.dt
    AF = mybir.ActivationFunctionType
    OP = mybir.AluOpType
    DR = mybir.MatmulPerfMode.DoubleRow

    DH = D // H
    assert DH == P, "layout assumes head dim == 128"
    ST = S // P           # token tiles
    KD = D // P           # D contraction tiles
    KD2 = KD // 2         # DoubleRow k-pairs over D
    KF = DFF // P         # DFF contraction tiles / ff1 out tiles
    NCD = min(512, D)     # fo chunk for out_proj/ff2 (psum-bank sized)
    NOD = D // NCD
    NCS = min(512, S)     # s chunk
    NOS = S // NCS
    GT = ST // 2          # phase-3 mt-group size
    inv_sqrt_dh = float(1.0 / np.sqrt(DH))
    OUT_SCALE = float(1.0 / (OSCALE * WSCALE))   # out_proj psum descale
    QK_SCALE = float(1.0 / WSCALE)               # q/k eviction descale
    V_SCALE = float(VSCALE / WSCALE)             # v eviction: VSCALE * (1/WSCALE)

    # ---- DRAM I/O ----
    src_d = nc.dram_tensor("src", [S, D], dt.float32, kind="ExternalInput")
    srcb_d = nc.dram_tensor("srcb", [S, D], dt.float32, kind="ExternalInput")  # src + out_proj_b
    w_in_d = nc.dram_tensor("w_in_blk", [3 * KD, P, KD, P], dt.float8e4, kind="ExternalInput")
    wo_d = nc.dram_tensor("wo_pair", [NOD, KD2, P, 2, NCD], dt.float8e4, kind="ExternalInput")
    w1_d = nc.dram_tensor("w1_blk", [KF, P, KD, P], dt.bfloat16, kind="ExternalInput")
    w2_d = nc.dram_tensor("w2_chunk", [NOD, KF, P, NCD], dt.bfloat16, kind="ExternalInput")
    bin_d = nc.dram_tensor("bias_in", [P, 3 * KD], dt.float32, kind="ExternalInput")
    b1_d = nc.dram_tensor("b1_t", [P, KF], dt.float32, kind="ExternalInput")
    alpha_d = nc.dram_tensor("alpha_t", [P, KF], dt.float32, kind="ExternalInput")
    ab1_d = nc.dram_tensor("ab1_t", [P, KF], dt.float32, kind="ExternalInput")
    gam_d = nc.dram_tensor("gamma_t", [P, KF], dt.float32, kind="ExternalInput")
    n1w_d = nc.dram_tensor("n1w_bc", [P, D], dt.float32, kind="ExternalInput")
    n2w_d = nc.dram_tensor("n2w_bc", [P, D], dt.float32, kind="ExternalInput")
    b2_d = nc.dram_tensor("b2_bc", [P, D], dt.float32, kind="ExternalInput")
    out_d = nc.dram_tensor("out", [S, D], dt.float32, kind="ExternalOutput")
    # internal DRAM spills
    xb2_d = nc.dram_tensor("xb2_spill", [ST, P, D], dt.float32)  # x + b2
    h2_d = nc.dram_tensor("h2_spill", [KF, P, S], dt.bfloat16)

    with tile.TileContext(nc) as tc:
        # ---------- persistent constants (whole kernel) ----------
        cp = tc.alloc_tile_pool(name="consts", bufs=1)
        ident = cp.tile([P, P], dt.bfloat16)
        make_identity(nc, ident)
        identf = cp.tile([P, P], dt.float32)
        make_identity(nc, identf)
        half_kb = cp.tile([P, 1], dt.bfloat16)
        nc.any.memset(half_kb[:], 0.5)   # folds rb = 2/denom
        ones_1 = cp.tile([1, P], dt.bfloat16)
        nc.any.memset(ones_1[:], 1.0)
        ones_1f = cp.tile([1, P], dt.float32)
        nc.any.memset(ones_1f[:], 1.0)
        eshift = cp.tile([P, 1], dt.float32)
        nc.any.memset(eshift[:], -EXP_SHIFT)
        bin_sb = cp.tile([P, 3 * KD], dt.float32)
        nc.sync.dma_start(out=bin_sb[:], in_=bin_d.ap())
        b1_sb = cp.tile([P, KF], dt.float32)
        nc.sync.dma_start(out=b1_sb[:], in_=b1_d.ap())
        alpha_sb = cp.tile([P, KF], dt.float32)
        nc.sync.dma_start(out=alpha_sb[:], in_=alpha_d.ap())
        ab1_sb = cp.tile([P, KF], dt.float32)
        nc.sync.dma_start(out=ab1_sb[:], in_=ab1_d.ap())
        gam_sb = cp.tile([P, KF], dt.float32)
        nc.sync.dma_start(out=gam_sb[:], in_=gam_d.ap())
        c2_tok = cp.tile([P, ST], dt.float32)   # filled in phase 4
        sc2_bc = cp.tile([P, S], dt.float32)    # filled in phase 4
        # prefetch head 0's q block so phase 2 starts on x2T alone
        wpre0 = cp.tile([P, KD, P], dt.float8e4, name="wpre_0")
        nc.sync.dma_start(out=wpre0[:], in_=w_in_d.ap()[0])

        # ================= phase 1: rmsnorm1 + transpose (fp8) =================
        xp = tc.alloc_tile_pool(name="x2T_pool", bufs=1)
        x2T = xp.tile([P, KD, S], dt.float8e4)

        p1 = tc.alloc_tile_pool(name="p1", bufs=2)
        p1c = tc.alloc_tile_pool(name="p1c", bufs=1)
        p1ps = tc.alloc_tile_pool(name="p1ps", bufs=4, space="PSUM")
        n1w_sb = p1c.tile([P, D], dt.float32)
        nc.sync.dma_start(out=n1w_sb[:], in_=n1w_d.ap())
        for mt in range(ST):
            xt = p1.tile([P, D], dt.float32, tag="xt", bufs=8)
            nc.sync.dma_start(out=xt[:], in_=src_d.ap()[ts(mt, P), :])
            sq = p1.tile([P, D], dt.float32, tag="sq", bufs=3)
            ss = p1.tile([P, 1], dt.float32, tag="ss")
            nc.scalar.activation(sq[:], xt[:], AF.Square, accum_out=ss[:])
            ms = p1.tile([P, 1], dt.float32, tag="ms")
            nc.vector.tensor_scalar(ms[:], ss[:], 1.0 / D, EPS, op0=OP.mult, op1=OP.add)
            rt = p1.tile([P, 1], dt.float32, tag="rt")
            nc.scalar.activation(rt[:], ms[:], AF.Sqrt)
            rs = p1.tile([P, 1], dt.float32, tag="rs")
            nc.vector.reciprocal(rs[:], rt[:])
            x2 = p1.tile([P, D], dt.bfloat16, tag="x2", bufs=3)
            nc.vector.scalar_tensor_tensor(x2[:], xt[:], rs[:], n1w_sb[:], op0=OP.mult, op1=OP.mult)
            for ko in range(KD):
                pt = p1ps.tile([P, P], dt.bfloat16, tag="tp")
                nc.tensor.transpose(pt[:], x2[:, ts(ko, P)], ident[:])
                nc.vector.tensor_copy(x2T[:, ko, ts(mt, P)], pt[:])
        p1ps.release()
        p1c.release()
        p1.release()

        # ================= phase 2: fused in_proj (fp8 DR) + attention =================
        op_ = tc.alloc_tile_pool(name="oT_pool", bufs=1, side="right")
        oT_all = op_.tile([P, KD, S], dt.float8e4)
        # residual tiles for phase 3, seeded during phase 2 (DMA slack)
        p3x = tc.alloc_tile_pool(name="p3x", bufs=1, side="right")
        xg = [p3x.tile([P, D], dt.float32, tag=f"xg{i}", name=f"xg_{i}")
              for i in range(ST)]

        p2w = tc.alloc_tile_pool(name="p2w", bufs=2)
        p2 = tc.alloc_tile_pool(name="p2", bufs=2)
        p2a = tc.alloc_tile_pool(name="p2a", bufs=2, space="PSUM")
        p2b = tc.alloc_tile_pool(name="p2b", bufs=2, space="PSUM")
        p2c = tc.alloc_tile_pool(name="p2c", bufs=1, space="PSUM")

        def attn_tail(h, expT, vT, sum8):
            # softmax denominator: partition-reduce sum8 on PE, broadcast,
            # fast-reciprocal; then o^T = (sum_t v16^T @ exp^T) * (2/denom).
            # Emitted one head late so the PE never waits on ACT's exp.
            rb = p2.tile([P, S], dt.float32, tag="rb", name=f"rb_{h}")
            for sc in range(NOS):
                psr = p2c.tile([1, NCS], dt.float32, tag="den", name=f"psr_{h}_{sc}")
                nc.tensor.matmul(psr[:], half_kb[:], sum8[:, ts(sc, NCS)],
                                 start=True, stop=True)
                srow = p2.tile([1, NCS], dt.bfloat16, tag="srow", name=f"srow_{h}_{sc}")
                nc.scalar.activation(srow[:], psr[:], AF.Copy)
                psb = p2c.tile([P, NCS], dt.float32, tag="den", name=f"psb_{h}_{sc}")
                nc.tensor.matmul(psb[:], ones_1[:], srow[:], start=True, stop=True)
                nc.vector.reciprocal_approx_fast(rb[:, ts(sc, NCS)], psb[:])
            for sc in range(NOS):
                po = p2a.tile([P, NCS], dt.float32, tag="po", bufs=2, name=f"po_{h}_{sc}")
                for t2 in range(ST // 2):
                    nc.tensor.matmul(po[:], vT[:, 2 * t2:2 * t2 + 2, :],
                                     expT[:, 2 * t2:2 * t2 + 2, ts(sc, NCS)],
                                     start=(t2 == 0), stop=(t2 == ST // 2 - 1),
                                     perf_mode=DR)
                nc.vector.tensor_tensor(oT_all[:, h, ts(sc, NCS)], po[:], rb[:, ts(sc, NCS)],
                                        OP.mult)

        def qkv_block(h, j, mo, out_dtype, scale):
            # one projection (q/k/v) for head h: 2 s-chunks x 8 DR matmuls,
            # evicted on DVE (scale + bias) to keep ACT free for exp
            if h == 0 and j == 0:
                wblk = wpre0
            else:
                wblk = p2w.tile([P, KD, P], dt.float8e4, tag="wblk", bufs=4)
                nc.sync.dma_start(out=wblk[:], in_=w_in_d.ap()[mo])
            dest = p2.tile([P, S], out_dtype, tag=f"qkv{j}", name=f"qkv{j}_{h}")
            for sc in range(NOS):
                ps = p2a.tile([P, NCS], dt.float32, tag="mmps", bufs=2,
                              name=f"qkvps_{h}_{j}_{sc}")
                for k2 in range(KD2):
                    nc.tensor.matmul(ps[:], wblk[:, 2 * k2:2 * k2 + 2, :],
                                     x2T[:, 2 * k2:2 * k2 + 2, ts(sc, NCS)],
                                     start=(k2 == 0), stop=(k2 == KD2 - 1),
                                     perf_mode=DR)
                nc.vector.tensor_scalar(dest[:, ts(sc, NCS)], ps[:], scale,
                                        bin_sb[:, mo:mo + 1], op0=OP.mult, op1=OP.add)
            return dest

        prev = None
        for h in range(H):
            if 4 <= h < 4 + ST:  # spread the 8 residual seeds across heads
                nc.sync.dma_start(out=xg[h - 4][:], in_=srcb_d.ap()[ts(h - 4, P), :])
            k = qkv_block(h, 1, KD + h, dt.bfloat16, QK_SCALE)
            q = qkv_block(h, 0, h, dt.bfloat16, QK_SCALE)
            # prev head's denominator + attn@v fill the PE while this head's
            # exp evictions (ACT) drain into the next head's DR stream
            if prev is not None:
                attn_tail(*prev)
            v = qkv_block(h, 2, 2 * KD + h, dt.bfloat16, V_SCALE)
            # v^T via PE transpose (bf16 -> fp8 on the copy out)
            vT = p2.tile([P, ST, P], dt.float8e4, tag="vT", name=f"vT_{h}")
            for tt in range(ST):
                pt = p2c.tile([P, P], dt.bfloat16, tag="vtp", name=f"vtp_{h}_{tt}")
                nc.tensor.transpose(pt[:], v[:, ts(tt, P)], ident[:])
                nc.vector.tensor_copy(vT[:, tt, :], pt[:])
            # scores^T (bf16) -> exp - EXP_SHIFT (fp8); emitted last so the
            # ACT exps overlap the next head's k/q DR stream
            expT = p2.tile([P, ST, S], dt.float8e4, tag="expT", name=f"expT_{h}")
            for tt in range(ST):
                for sc in range(NOS):
                    ps = p2b.tile([P, NCS], dt.float32, tag="scps", name=f"scps_{h}_{tt}_{sc}")
                    nc.tensor.matmul(ps[:], k[:, ts(tt, P)], q[:, ts(sc, NCS)],
                                     start=True, stop=True)
                    nc.scalar.activation(expT[:, tt, ts(sc, NCS)], ps[:], AF.Exp,
                                         scale=inv_sqrt_dh, bias=eshift[:])
            # denominator partial: tree-add the 8 expT t-tiles (DVE + GpSimd)
            t01 = p2.tile([P, S], dt.bfloat16, tag="t01", name=f"t01_{h}")
            nc.vector.tensor_tensor(t01[:], expT[:, 0, :], expT[:, 1, :], OP.add)
            t23 = p2.tile([P, S], dt.bfloat16, tag="t23", name=f"t23_{h}")
            nc.gpsimd.tensor_tensor(t23[:], expT[:, 2, :], expT[:, 3, :], OP.add)
            t45 = p2.tile([P, S], dt.bfloat16, tag="t45", name=f"t45_{h}")
            nc.vector.tensor_tensor(t45[:], expT[:, 4, :], expT[:, 5, :], OP.add)
            t67 = p2.tile([P, S], dt.bfloat16, tag="t67", name=f"t67_{h}")
            nc.gpsimd.tensor_tensor(t67[:], expT[:, 6, :], expT[:, 7, :], OP.add)
            ta = p2.tile([P, S], dt.bfloat16, tag="ta", name=f"ta_{h}")
            nc.vector.tensor_tensor(ta[:], t01[:], t23[:], OP.add)
            tb = p2.tile([P, S], dt.bfloat16, tag="tb", name=f"tb_{h}")
            nc.gpsimd.tensor_tensor(tb[:], t45[:], t67[:], OP.add)
            sum8 = p2.tile([P, S], dt.bfloat16, tag="sum8", name=f"sum8_{h}")
            nc.vector.tensor_tensor(sum8[:], ta[:], tb[:], OP.add)
            prev = (h, expT, vT, sum8)
        attn_tail(*prev)
        p2c.release()
        p2b.release()
        p2a.release()
        p2.release()
        p2w.release()
        xp.release()  # x2T dead

        # ===== phase 3: out_proj (fp8 DR) + residual + rmsnorm2 + quant =====
        # Two mt-groups: group 1's matmuls overlap group 0's rmsnorm/quant
        # chain, and ff1's first s-chunk needs exactly group 0's x2qT slices.
        qp = tc.alloc_tile_pool(name="x2qT_pool", bufs=1)
        x2qT = qp.tile([P, KD, S], dt.bfloat16)
        c1_bc = qp.tile([P, S], dt.float32)

        p3c = tc.alloc_tile_pool(name="p3c", bufs=1)
        p3q = tc.alloc_tile_pool(name="p3q", bufs=2)
        p3 = tc.alloc_tile_pool(name="p3", bufs=3)
        ps34 = tc.alloc_tile_pool(name="ps34", bufs=1, space="PSUM")
        n2w_sb = p3c.tile([P, D], dt.float32)
        nc.sync.dma_start(out=n2w_sb[:], in_=n2w_d.ap())
        b2_sb = p3c.tile([P, D], dt.float32)
        nc.sync.dma_start(out=b2_sb[:], in_=b2_d.ap())
        c1row = p3c.tile([1, S], dt.float32)

        def outproj_group(g):
            mts = range(g * GT, (g + 1) * GT)
            for no in range(NOD):
                psy = [ps34.tile([P, NCD], dt.float32, tag="acc", bufs=4,
                                 name=f"psy3_{g}_{no}_{i}") for i in range(GT)]
                for k2 in range(KD2):
                    wop = p3.tile([P, 2, NCD], dt.float8e4, tag="wop", bufs=6)
                    nc.sync.dma_start(out=wop[:], in_=wo_d.ap()[no, k2])
                    for i, mt in enumerate(mts):
                        nc.tensor.matmul(psy[i][:], oT_all[:, 2 * k2:2 * k2 + 2, ts(mt, P)],
                                         wop[:], start=(k2 == 0), stop=(k2 == KD2 - 1),
                                         perf_mode=DR)
                for i, mt in enumerate(mts):
                    ch = ts(no, NCD)
                    # x += o (descaled); single DVE op keeps the PSUM bank hot
                    nc.vector.scalar_tensor_tensor(xg[mt][:, ch], psy[i][:], OUT_SCALE,
                                                   xg[mt][:, ch], op0=OP.mult, op1=OP.add)
                    xo = p3.tile([P, NCD], dt.float32, tag="xo", bufs=3, name=f"xo_{no}_{mt}")
                    nc.vector.tensor_tensor(xo[:], xg[mt][:, ch], b2_sb[:, ch], OP.add)
                    nc.sync.dma_start(out=xb2_d.ap()[mt][:, ch], in_=xo[:])

        def quant_compute(mt):
            # rmsnorm2 + per-token absmax + int8 round (ACT/DVE only, no PE)
            xt = xg[mt]
            sq = p3q.tile([P, D], dt.float32, tag="scr", bufs=2, name=f"sq3_{mt}")
            ss = p3q.tile([P, 1], dt.float32, tag="ss", bufs=4)
            nc.scalar.activation(sq[:], xt[:], AF.Square, accum_out=ss[:])
            ms = p3q.tile([P, 1], dt.float32, tag="ms", bufs=4)
            nc.vector.tensor_scalar(ms[:], ss[:], 1.0 / D, EPS, op0=OP.mult, op1=OP.add)
            rt = p3q.tile([P, 1], dt.float32, tag="rt", bufs=4)
            nc.scalar.activation(rt[:], ms[:], AF.Sqrt)
            rs = p3q.tile([P, 1], dt.float32, tag="rs", bufs=4)
            nc.vector.reciprocal(rs[:], rt[:])
            x2 = p3q.tile([P, D], dt.float32, tag="x2", bufs=2)
            nc.vector.scalar_tensor_tensor(x2[:], xt[:], rs[:], n2w_sb[:],
                                           op0=OP.mult, op1=OP.mult)
            mx = p3q.tile([P, 1], dt.float32, tag="mx", bufs=4)
            nc.vector.tensor_reduce(mx[:], x2[:], axis=mybir.AxisListType.X,
                                    op=OP.max, apply_absolute_value=True)
            mcl = p3q.tile([P, 1], dt.float32, tag="mcl", bufs=4)
            nc.vector.tensor_scalar(mcl[:], mx[:], 1e-5, None, op0=OP.max)
            rc = p3q.tile([P, 1], dt.float32, tag="rc", bufs=4)
            nc.vector.reciprocal(rc[:], mcl[:])
            sc1 = p3q.tile([P, 1], dt.float32, tag="sc1", bufs=4)
            nc.vector.tensor_scalar(sc1[:], rc[:], 127.0, None, op0=OP.mult)
            c1c = p3q.tile([P, 1], dt.float32, tag="c1c", bufs=4)
            nc.vector.tensor_scalar(c1c[:], mcl[:], ws1 / 127.0, None, op0=OP.mult)
            nc.sync.dma_start(out=c1row[0:1, ts(mt, P)], in_=c1c[:])
            t1 = p3q.tile([P, D], dt.float32, tag="scr", bufs=2, name=f"t1_{mt}")
            nc.vector.tensor_scalar(t1[:], x2[:], sc1[:], MAGIC, op0=OP.mult, op1=OP.add)
            xq = p3q.tile([P, D], dt.bfloat16, tag="xq", bufs=4)
            nc.vector.tensor_scalar(xq[:], t1[:], MAGIC, None, op0=OP.subtract)
            return xq

        def quant_transpose(mt, xq):
            for ko in range(KD):
                pt = ps34.tile([P, P], dt.bfloat16, tag="tp", bufs=2)
                nc.tensor.transpose(pt[:], xq[:, ts(ko, P)], ident[:])
                nc.vector.tensor_copy(x2qT[:, ko, ts(mt, P)], pt[:])

        def c1_broadcast(g):
            pb = ps34.tile([P, NCS], dt.float32, tag="pb", bufs=2, name=f"pb3_{g}")
            nc.tensor.matmul(pb[:], ones_1f[:], c1row[:, ts(g, NCS)],
                             start=True, stop=True)
            nc.vector.tensor_copy(c1_bc[:, ts(g, NCS)], pb[:])

        # pipeline: group 1's matmuls cover group 0's quant chain; ff1's first
        # s-chunk (which needs only group 0) covers group 1's quant chain
        outproj_group(0)
        xqs0 = [quant_compute(mt) for mt in range(GT)]
        outproj_group(1)
        for mt in range(GT):
            quant_transpose(mt, xqs0[mt])
        c1_broadcast(0)
        xqs1 = [quant_compute(mt) for mt in range(GT, ST)]
        p3x.release()   # xg dead
        op_.release()   # oT_all dead

        # ================= phase 4: ff1 + snake + h2 spill + absmax =================
        p4 = tc.alloc_tile_pool(name="p4", bufs=3)
        p4m = tc.alloc_tile_pool(name="p4m", bufs=1)
        M_acc = p4m.tile([P, S], dt.float32)
        nc.any.memset(M_acc[:], 0.0)

        def ff1_block(sc, mo):
            wblk = p4.tile([P, KD, P], dt.bfloat16, tag="wblk")
            nc.sync.dma_start(out=wblk[:], in_=w1_d.ap()[mo])
            ph = ps34.tile([P, NCS], dt.float32, tag="acc", bufs=4, name=f"ph_{sc}_{mo}")
            for ko in range(KD):
                nc.tensor.matmul(ph[:], wblk[:, ko, :], x2qT[:, ko, ts(sc, NCS)],
                                 start=(ko == 0), stop=(ko == KD - 1))
            ch = ts(sc, NCS)
            t_ = p4.tile([P, NCS], dt.float32, tag="t_", name=f"t_{sc}_{mo}")
            nc.vector.tensor_tensor(t_[:], ph[:], c1_bc[:, ch], OP.mult)
            s_ = p4.tile([P, NCS], dt.float32, tag="s_", name=f"s_{sc}_{mo}")
            nc.scalar.activation(s_[:], t_[:], AF.Sin,
                                 scale=alpha_sb[:, mo:mo + 1], bias=ab1_sb[:, mo:mo + 1])
            h_ = p4.tile([P, NCS], dt.float32, tag="h_", name=f"h_{sc}_{mo}")
            nc.scalar.activation(h_[:], t_[:], AF.Identity, bias=b1_sb[:, mo:mo + 1])
            sq_ = p4.tile([P, NCS], dt.float32, tag="sq_", name=f"sq_{sc}_{mo}")
            nc.scalar.activation(sq_[:], s_[:], AF.Square)
            h2_ = p4.tile([P, NCS], dt.bfloat16, tag="h2_", name=f"h2_{sc}_{mo}")
            nc.vector.scalar_tensor_tensor(h2_[:], sq_[:], gam_sb[:, mo:mo + 1], h_[:],
                                           op0=OP.mult, op1=OP.add)
            nc.sync.dma_start(out=h2_d.ap()[mo][:, ch], in_=h2_[:])
            am_ = p4.tile([P, NCS], dt.float32, tag="am_", name=f"am_{sc}_{mo}")
            nc.scalar.activation(am_[:], h2_[:], AF.Abs)
            nc.vector.tensor_tensor(M_acc[:, ch], M_acc[:, ch], am_[:], OP.max)

        FF1_PRE = 8   # sc0 blocks emitted before group 1's transposes
        for mo in range(FF1_PRE):
            ff1_block(0, mo)
        for i, mt in enumerate(range(GT, ST)):
            quant_transpose(mt, xqs1[i])
        c1_broadcast(1)
        for mo in range(FF1_PRE, KF):
            ff1_block(0, mo)
        for mo in range(KF):
            ff1_block(1, mo)

        # cross-partition absmax via PE transpose + free-axis reduce
        m_tok = p4m.tile([P, ST], dt.float32)
        for c in range(ST):
            pmt = ps34.tile([P, P], dt.float32, tag="tp", bufs=2, name=f"pmt_{c}")
            nc.tensor.transpose(pmt[:], M_acc[:, ts(c, P)], identf[:])
            nc.vector.tensor_reduce(m_tok[:, c:c + 1], pmt[:], axis=mybir.AxisListType.X, op=OP.max)
        mcl2 = p4m.tile([P, ST], dt.float32)
        nc.vector.tensor_scalar(mcl2[:], m_tok[:], 1e-5, None, op0=OP.max)
        rc2 = p4m.tile([P, ST], dt.float32)
        nc.vector.reciprocal(rc2[:], mcl2[:])
        sc2_tok = p4m.tile([P, ST], dt.float32)
        nc.vector.tensor_scalar(sc2_tok[:], rc2[:], 127.0, None, op0=OP.mult)
        nc.vector.tensor_scalar(c2_tok[:], mcl2[:], ws2 / 127.0, None, op0=OP.mult)
        sc2row = p4m.tile([1, S], dt.float32)
        for mt in range(ST):
            nc.sync.dma_start(out=sc2row[0:1, ts(mt, P)], in_=sc2_tok[:, mt:mt + 1])
        for sc in range(NOS):
            pb = ps34.tile([P, NCS], dt.float32, tag="pb", bufs=2, name=f"pb4_{sc}")
            nc.tensor.matmul(pb[:], ones_1f[:], sc2row[:, ts(sc, NCS)], start=True, stop=True)
            nc.vector.tensor_copy(sc2_bc[:, ts(sc, NCS)], pb[:])
        p4m.release()
        p4.release()
        ps34.release()
        p3.release()
        p3q.release()
        p3c.release()
        qp.release()  # x2qT, c1_bc dead

        # ===== phase 5+6: ff2, with h2 quantization fused into the first no pass =====
        q2p = tc.alloc_tile_pool(name="xq2_pool", bufs=1)
        xq2 = q2p.tile([P, KF, S], dt.bfloat16)
        p6 = tc.alloc_tile_pool(name="p6", bufs=3)
        p6ps = tc.alloc_tile_pool(name="p6ps", bufs=1, space="PSUM")
        for no in range(NOD):
            psy = [p6ps.tile([P, NCD], dt.float32, tag=f"y{mt}", name=f"psy6_{no}_{mt}")
                   for mt in range(ST)]
            xchs = []
            for mt in range(ST):  # prefetch residual chunks
                xch = p6.tile([P, NCD], dt.float32, tag="xch", bufs=ST,
                              name=f"xch6_{no}_{mt}")
                nc.sync.dma_start(out=xch[:], in_=xb2_d.ap()[mt][:, ts(no, NCD)])
                xchs.append(xch)
            for ko in range(KF):
                if no == 0:
                    # quantize h2[ko] -> exact ints in bf16, just ahead of first use
                    h2t = p6.tile([P, S], dt.bfloat16, tag="h2t", bufs=3)
                    nc.sync.dma_start(out=h2t[:], in_=h2_d.ap()[ko])
                    m1 = p6.tile([P, S], dt.float32, tag="m1", bufs=2)
                    nc.gpsimd.tensor_tensor(m1[:], h2t[:], sc2_bc[:], OP.mult)
                    nc.vector.tensor_scalar(xq2[:, ko, :], m1[:], MAGIC, MAGIC,
                                            op0=OP.add, op1=OP.subtract)
                wch = p6.tile([P, NCD], dt.bfloat16, tag="wch", bufs=4)
                nc.sync.dma_start(out=wch[:], in_=w2_d.ap()[no, ko])
                for mt in range(ST):
                    nc.tensor.matmul(psy[mt][:], xq2[:, ko, ts(mt, P)], wch[:],
                                     start=(ko == 0), stop=(ko == KF - 1))
            for mt in range(ST):
                # single-op evict: out = psum * c2[token] + (x + b2)
                oe = p6.tile([P, NCD], dt.float32, tag="oe", bufs=4, name=f"oe_{no}_{mt}")
                nc.vector.scalar_tensor_tensor(oe[:], psy[mt][:], c2_tok[:, mt:mt + 1],
                                               xchs[mt][:], op0=OP.mult, op1=OP.add)
                nc.sync.dma_start(out=out_d.ap()[ts(mt, P), ts(no, NCD)], in_=oe[:])
        p6ps.release()
        p6.release()
        q2p.release()
        cp.release()
    return nc


# ---------------------------------------------------------------- driver

def _get_compiled(key, S, D, H, DFF, ws1, ws2):
    if key in _CACHE:
        return _CACHE[key]
    from concourse import bacc

    nc = bacc.Bacc("TRN2", target_bir_lowering=False, debug=False, num_devices=NCORES)
    build_program(nc, S=S, D=D, H=H, DFF=DFF, ws1=ws1, ws2=ws2)
    nc.compile()
    _CACHE[key] = nc
    return nc


def make_in_maps(inputs):
    src = np.asarray(inputs["src"], dtype=np.float32)
    B, S, D = src.shape
    H = H_FULL
    DFF = inputs["ff1_w"].shape[0]
    arrays, ws1, ws2 = _prep_arrays(inputs, S, D, H, DFF)
    srcb = src + np.asarray(inputs["out_proj_b"], dtype=np.float32)[None, None, :]
    in_maps = []
    for c in range(NCORES):
        m = dict(arrays)
        m["src"] = np.ascontiguousarray(src[c])
        m["srcb"] = np.ascontiguousarray(srcb[c])
        in_maps.append(m)
    return in_maps, (S, D, H, DFF, ws1, ws2)


def kernel(**inputs):
    from concourse.bass_utils import run_bass_kernel_spmd

    in_maps, (S, D, H, DFF, ws1, ws2) = make_in_maps(inputs)
    assert np.asarray(inputs["src"]).shape[0] == NCORES
    nc = _get_compiled(("full", S, D, H, DFF, ws1, ws2), S, D, H, DFF, ws1, ws2)
    res = run_bass_kernel_spmd(nc, in_maps, core_ids=list(range(NCORES)))
    out = np.stack([res.results[c]["out"] for c in range(NCORES)], axis=0)
    return out.astype(np.float32)


# revision 31
# speedup vs baseline: 1.0467x; 1.0467x over previous
"""BitTransformerEncoderLayer on 8 TRN2 NeuronCores.

Strategy: pure data parallelism over batch (B=8 == n_cores); no collectives.
v2: fp8e4 DoubleRow matmuls (2x PE rate) for in_proj / out_proj / attn@v /
softmax-denominator; scores and the BitLinear FFN matmuls stay bf16 (the FFN
runs exact integer arithmetic in bf16 — fp8 would round ints > 16 and blow the
error budget). h2 spills to DRAM as bf16. Softmax denominators via DVE tree +
one ones-matmul; reciprocal_approx_fast for 1/denom.

kernel(**inputs) takes the FULL unsharded inputs and returns the FULL output.
"""

import numpy as np

P = 128
EPS = 1e-8
MAGIC = 12582912.0  # 1.5 * 2**23: fp32 add/sub rounds to nearest-even integer
NCORES = 8
EXP_SHIFT = 2.0     # exp(scores - 2): max exp ~125 < 240 (TRN e4m3 max)
WSCALE = 64.0       # fp8 weight pre-scale for in_proj/out_proj
VSCALE = 16.0       # fp8 v pre-scale
OSCALE = 32.0       # fp8 o (attn out) pre-scale

# Problem dims (hardcoded per spec)
B_FULL, S_FULL, D_FULL, H_FULL, DFF_FULL = 8, 1024, 2048, 16, 8192

_CACHE = {}


# ---------------------------------------------------------------- host prep

def _quant_w(w):
    scale = np.maximum(np.mean(np.abs(w), dtype=np.float32), np.float32(1e-5))
    q = np.clip(np.round(w / scale), -1.0, 1.0).astype(np.float32)
    return q, float(scale)


def _lhsT_blocks(w):
    """w [M, K] -> [M/P, P(k), K/P, P(m)]; [mo, :, ko, :] = w-block(mo, ko).T"""
    M, K = w.shape
    t = w.reshape(M // P, P, K // P, P)  # [mo, pm, ko, pk]
    return np.ascontiguousarray(t.transpose(0, 3, 2, 1))


def _rhs_chunks(w, nch):
    """w [N, K] -> [N/nch, K/P, P, nch]; [no, ko, p, j] = w[no*nch+j, ko*P+p]"""
    N, K = w.shape
    t = w.reshape(N // nch, nch, K // P, P)  # [no, j, ko, p]
    return np.ascontiguousarray(t.transpose(0, 2, 3, 1))


def _rhs_pair_chunks(w, nch):
    """w [N, K] -> [N/nch, K/2P, P, 2, nch]; [no, k2, p, i, j] = w[no*nch+j, (2*k2+i)*P+p]"""
    N, K = w.shape
    t = w.reshape(N // nch, nch, K // (2 * P), 2, P)  # [no, j, k2, i, p]
    return np.ascontiguousarray(t.transpose(0, 2, 4, 3, 1))


def _per_part(v):
    """[M] -> [P, M/P]; out[p, mo] = v[mo*P + p]"""
    return np.ascontiguousarray(v.reshape(-1, P).T)


def _bcast_row(v):
    return np.ascontiguousarray(np.broadcast_to(v[None, :], (P, v.shape[0])))


def _prep_arrays(inputs, S, D, H, DFF):
    import ml_dtypes

    bf16 = ml_dtypes.bfloat16
    f8 = ml_dtypes.float8_e4m3fn
    f32 = np.float32
    g = lambda k: np.asarray(inputs[k], dtype=f32)

    w1q, ws1 = _quant_w(g("ff1_w"))   # [DFF, D]
    w2q, ws2 = _quant_w(g("ff2_w"))   # [D, DFF]
    ncd = min(512, D)

    bias_in = _per_part(g("in_proj_b")).astype(f32)  # [P, 3D/P]
    bias_in[:, 2 * (D // P):] *= VSCALE  # v eviction is pre-scaled by VSCALE

    arrays = {
        "w_in_blk": _lhsT_blocks(g("in_proj_w") * WSCALE).astype(f8),  # [3D/P, P, D/P, P]
        "wo_pair": _rhs_pair_chunks(g("out_proj_w") * WSCALE, ncd).astype(f8),
        "w1_blk": _lhsT_blocks(w1q).astype(bf16),                # [DFF/P, P, D/P, P]
        "w2_chunk": _rhs_chunks(w2q, ncd).astype(bf16),          # [D/ncd, DFF/P, P, ncd]
        "bias_in": bias_in,                                      # [P, 3D/P]
        "b1_t": _per_part(g("ff1_b")).astype(f32),               # [P, DFF/P]
        "alpha_t": _per_part(g("alpha")).astype(f32),
        "ab1_t": _per_part((g("alpha") * g("ff1_b")).astype(f32)),
        "gamma_t": _per_part((1.0 / (g("beta") + np.float32(1e-9))).astype(f32)),
        "n1w_bc": _bcast_row(g("norm1_w")).astype(f32),          # [P, D]
        "n2w_bc": _bcast_row(g("norm2_w")).astype(f32),
        "b2_bc": _bcast_row(g("ff2_b")).astype(f32),
    }
    return arrays, ws1, ws2


# ---------------------------------------------------------------- device program

def build_program(nc, *, S, D, H, DFF, ws1, ws2):
    import concourse.mybir as mybir
    import concourse.tile as tile
    from concourse.bass import ts
    from concourse.masks import make_identity

    dt = mybir.dt
    AF = mybir.ActivationFunctionType
    OP = mybir.AluOpType
    DR = mybir.MatmulPerfMode.DoubleRow

    DH = D // H
    assert DH == P, "layout assumes head dim == 128"
    ST = S // P           # token tiles
    KD = D // P           # D contraction tiles
    KD2 = KD // 2         # DoubleRow k-pairs over D
    KF = DFF // P         # DFF contraction tiles / ff1 out tiles
    NCD = min(512, D)     # fo chunk for out_proj/ff2 (psum-bank sized)
    NOD = D // NCD
    NCS = min(512, S)     # s chunk
    NOS = S // NCS
    GT = ST // 2          # phase-3 mt-group size
    inv_sqrt_dh = float(1.0 / np.sqrt(DH))
    OUT_SCALE = float(1.0 / (OSCALE * WSCALE))   # out_proj psum descale
    QK_SCALE = float(1.0 / WSCALE)               # q/k eviction descale
    V_SCALE = float(VSCALE / WSCALE)             # v eviction: VSCALE * (1/WSCALE)

    # ---- DRAM I/O ----
    src_d = nc.dram_tensor("src", [S, D], dt.float32, kind="ExternalInput")
    srcb_d = nc.dram_tensor("srcb", [S, D], dt.float32, kind="ExternalInput")  # src + out_proj_b
    w_in_d = nc.dram_tensor("w_in_blk", [3 * KD, P, KD, P], dt.float8e4, kind="ExternalInput")
    wo_d = nc.dram_tensor("wo_pair", [NOD, KD2, P, 2, NCD], dt.float8e4, kind="ExternalInput")
    w1_d = nc.dram_tensor("w1_blk", [KF, P, KD, P], dt.bfloat16, kind="ExternalInput")
    w2_d = nc.dram_tensor("w2_chunk", [NOD, KF, P, NCD], dt.bfloat16, kind="ExternalInput")
    bin_d = nc.dram_tensor("bias_in", [P, 3 * KD], dt.float32, kind="ExternalInput")
    b1_d = nc.dram_tensor("b1_t", [P, KF], dt.float32, kind="ExternalInput")
    alpha_d = nc.dram_tensor("alpha_t", [P, KF], dt.float32, kind="ExternalInput")
    ab1_d = nc.dram_tensor("ab1_t", [P, KF], dt.float32, kind="ExternalInput")
    gam_d = nc.dram_tensor("gamma_t", [P, KF], dt.float32, kind="ExternalInput")
    n1w_d = nc.dram_tensor("n1w_bc", [P, D], dt.float32, kind="ExternalInput")
    n2w_d = nc.dram_tensor("n2w_bc", [P, D], dt.float32, kind="ExternalInput")
    b2_d = nc.dram_tensor("b2_bc", [P, D], dt.float32, kind="ExternalInput")
    out_d = nc.dram_tensor("out", [S, D], dt.float32, kind="ExternalOutput")
    # internal DRAM spills
    xb2_d = nc.dram_tensor("xb2_spill", [ST, P, D], dt.float32)  # x + b2
    h2_d = nc.dram_tensor("h2_spill", [KF, P, S], dt.bfloat16)

    with tile.TileContext(nc) as tc:
        # ---------- persistent constants (whole kernel) ----------
        cp = tc.alloc_tile_pool(name="consts", bufs=1)
        ident = cp.tile([P, P], dt.bfloat16)
        make_identity(nc, ident)
        identf = cp.tile([P, P], dt.float32)
        make_identity(nc, identf)
        half_kb = cp.tile([P, 1], dt.bfloat16)
        nc.any.memset(half_kb[:], 0.5)   # folds rb = 2/denom
        ones_1 = cp.tile([1, P], dt.bfloat16)
        nc.any.memset(ones_1[:], 1.0)
        ones_1f = cp.tile([1, P], dt.float32)
        nc.any.memset(ones_1f[:], 1.0)
        eshift = cp.tile([P, 1], dt.float32)
        nc.any.memset(eshift[:], -EXP_SHIFT)
        bin_sb = cp.tile([P, 3 * KD], dt.float32)
        b1_sb = cp.tile([P, KF], dt.float32)
        alpha_sb = cp.tile([P, KF], dt.float32)
        ab1_sb = cp.tile([P, KF], dt.float32)
        gam_sb = cp.tile([P, KF], dt.float32)
        c2_tok = cp.tile([P, ST], dt.float32)   # filled in phase 4
        sc2_bc = cp.tile([P, S], dt.float32)    # filled in phase 4
        wpre0 = cp.tile([P, KD, P], dt.float8e4, name="wpre_0")

        # ================= phase 1: rmsnorm1 + transpose (fp8) =================
        xp = tc.alloc_tile_pool(name="x2T_pool", bufs=1)
        x2T = xp.tile([P, KD, S], dt.float8e4)

        p1 = tc.alloc_tile_pool(name="p1", bufs=2)
        p1c = tc.alloc_tile_pool(name="p1c", bufs=1)
        p1ps = tc.alloc_tile_pool(name="p1ps", bufs=4, space="PSUM")
        # DMA issue order: n1w + src tiles first (they gate the critical
        # chain), head-0 weights next, small phase-2/4 consts last
        n1w_sb = p1c.tile([P, D], dt.float32)
        nc.sync.dma_start(out=n1w_sb[:], in_=n1w_d.ap())
        for mt in range(ST):
            xt = p1.tile([P, D], dt.float32, tag="xt", bufs=8)
            nc.sync.dma_start(out=xt[:], in_=src_d.ap()[ts(mt, P), :])
            sq = p1.tile([P, D], dt.float32, tag="sq", bufs=3)
            ss = p1.tile([P, 1], dt.float32, tag="ss")
            nc.scalar.activation(sq[:], xt[:], AF.Square, accum_out=ss[:])
            ms = p1.tile([P, 1], dt.float32, tag="ms")
            nc.vector.tensor_scalar(ms[:], ss[:], 1.0 / D, EPS, op0=OP.mult, op1=OP.add)
            rt = p1.tile([P, 1], dt.float32, tag="rt")
            nc.scalar.activation(rt[:], ms[:], AF.Sqrt)
            rs = p1.tile([P, 1], dt.float32, tag="rs")
            nc.vector.reciprocal(rs[:], rt[:])
            x2 = p1.tile([P, D], dt.bfloat16, tag="x2", bufs=3)
            nc.vector.scalar_tensor_tensor(x2[:], xt[:], rs[:], n1w_sb[:], op0=OP.mult, op1=OP.mult)
            for ko in range(KD):
                pt = p1ps.tile([P, P], dt.bfloat16, tag="tp")
                nc.tensor.transpose(pt[:], x2[:, ts(ko, P)], ident[:])
                nc.vector.tensor_copy(x2T[:, ko, ts(mt, P)], pt[:])
            if mt == 0:  # src mt0..7 + n1w issued; now the head-0 weights
                nc.sync.dma_start(out=wpre0[:], in_=w_in_d.ap()[0])
        nc.sync.dma_start(out=bin_sb[:], in_=bin_d.ap())
        nc.sync.dma_start(out=b1_sb[:], in_=b1_d.ap())
        nc.sync.dma_start(out=alpha_sb[:], in_=alpha_d.ap())
        nc.sync.dma_start(out=ab1_sb[:], in_=ab1_d.ap())
        nc.sync.dma_start(out=gam_sb[:], in_=gam_d.ap())
        p1ps.release()
        p1c.release()
        p1.release()

        # ================= phase 2: fused in_proj (fp8 DR) + attention =================
        op_ = tc.alloc_tile_pool(name="oT_pool", bufs=1, side="right")
        oT_all = op_.tile([P, KD, S], dt.float8e4)
        # residual tiles for phase 3, seeded during phase 2 (DMA slack)
        p3x = tc.alloc_tile_pool(name="p3x", bufs=1, side="right")
        xg = [p3x.tile([P, D], dt.float32, tag=f"xg{i}", name=f"xg_{i}")
              for i in range(ST)]

        p2w = tc.alloc_tile_pool(name="p2w", bufs=2)
        p2 = tc.alloc_tile_pool(name="p2", bufs=2)
        p2a = tc.alloc_tile_pool(name="p2a", bufs=2, space="PSUM")
        p2b = tc.alloc_tile_pool(name="p2b", bufs=2, space="PSUM")
        p2c = tc.alloc_tile_pool(name="p2c", bufs=1, space="PSUM")

        def attn_tail(h, expT, vT, sum8):
            # softmax denominator: partition-reduce sum8 on PE, broadcast,
            # fast-reciprocal; then o^T = (sum_t v16^T @ exp^T) * (2/denom).
            # Emitted one head late so the PE never waits on ACT's exp.
            rb = p2.tile([P, S], dt.float32, tag="rb", name=f"rb_{h}")
            for sc in range(NOS):
                psr = p2c.tile([1, NCS], dt.float32, tag="den", name=f"psr_{h}_{sc}")
                nc.tensor.matmul(psr[:], half_kb[:], sum8[:, ts(sc, NCS)],
                                 start=True, stop=True)
                srow = p2.tile([1, NCS], dt.bfloat16, tag="srow", name=f"srow_{h}_{sc}")
                nc.scalar.activation(srow[:], psr[:], AF.Copy)
                psb = p2c.tile([P, NCS], dt.float32, tag="den", name=f"psb_{h}_{sc}")
                nc.tensor.matmul(psb[:], ones_1[:], srow[:], start=True, stop=True)
                nc.vector.reciprocal_approx_fast(rb[:, ts(sc, NCS)], psb[:])
            for sc in range(NOS):
                po = p2a.tile([P, NCS], dt.float32, tag="po", bufs=2, name=f"po_{h}_{sc}")
                for t2 in range(ST // 2):
                    nc.tensor.matmul(po[:], vT[:, 2 * t2:2 * t2 + 2, :],
                                     expT[:, 2 * t2:2 * t2 + 2, ts(sc, NCS)],
                                     start=(t2 == 0), stop=(t2 == ST // 2 - 1),
                                     perf_mode=DR)
                nc.vector.tensor_tensor(oT_all[:, h, ts(sc, NCS)], po[:], rb[:, ts(sc, NCS)],
                                        OP.mult)

        def qkv_block(h, j, mo, out_dtype, scale):
            # one projection (q/k/v) for head h: 2 s-chunks x 8 DR matmuls,
            # evicted on DVE (scale + bias) to keep ACT free for exp
            if h == 0 and j == 0:
                wblk = wpre0
            else:
                wblk = p2w.tile([P, KD, P], dt.float8e4, tag="wblk", bufs=4)
                nc.sync.dma_start(out=wblk[:], in_=w_in_d.ap()[mo])
            dest = p2.tile([P, S], out_dtype, tag=f"qkv{j}", name=f"qkv{j}_{h}")
            for sc in range(NOS):
                ps = p2a.tile([P, NCS], dt.float32, tag="mmps", bufs=2,
                              name=f"qkvps_{h}_{j}_{sc}")
                for k2 in range(KD2):
                    nc.tensor.matmul(ps[:], wblk[:, 2 * k2:2 * k2 + 2, :],
                                     x2T[:, 2 * k2:2 * k2 + 2, ts(sc, NCS)],
                                     start=(k2 == 0), stop=(k2 == KD2 - 1),
                                     perf_mode=DR)
                nc.vector.tensor_scalar(dest[:, ts(sc, NCS)], ps[:], scale,
                                        bin_sb[:, mo:mo + 1], op0=OP.mult, op1=OP.add)
            return dest

        prev = None
        for h in range(H):
            if 4 <= h < 4 + ST:  # spread the 8 residual seeds across heads
                nc.sync.dma_start(out=xg[h - 4][:], in_=srcb_d.ap()[ts(h - 4, P), :])
            k = qkv_block(h, 1, KD + h, dt.bfloat16, QK_SCALE)
            q = qkv_block(h, 0, h, dt.bfloat16, QK_SCALE)
            # prev head's denominator + attn@v fill the PE while this head's
            # exp evictions (ACT) drain into the next head's DR stream
            if prev is not None:
                attn_tail(*prev)
            # v projection woven between the scores matmuls: the 450ns ACT exp
            # evictions pace the 2-bank scps rotation, so pure scores bursts
            # would stall the PE ~50% — the v DRs fill those slots
            mo_v = 2 * KD + h
            wblk_v = p2w.tile([P, KD, P], dt.float8e4, tag="wblk", bufs=4)
            nc.sync.dma_start(out=wblk_v[:], in_=w_in_d.ap()[mo_v])
            v = p2.tile([P, S], dt.bfloat16, tag="qkv2", name=f"qkv2_{h}")
            expT = p2.tile([P, ST, S], dt.float8e4, tag="expT", name=f"expT_{h}")
            vps = None
            for i in range(2 * ST):  # 16 weave slots
                tt, sc = divmod(i, NOS)
                ps = p2b.tile([P, NCS], dt.float32, tag="scps", name=f"scps_{h}_{tt}_{sc}")
                nc.tensor.matmul(ps[:], k[:, ts(tt, P)], q[:, ts(sc, NCS)],
                                 start=True, stop=True)
                nc.scalar.activation(expT[:, tt, ts(sc, NCS)], ps[:], AF.Exp,
                                     scale=inv_sqrt_dh, bias=eshift[:])
                vsc, k2 = divmod(i, KD2)
                if k2 == 0:
                    vps = p2a.tile([P, NCS], dt.float32, tag="mmps", bufs=2,
                                   name=f"qkvps_{h}_2_{vsc}")
                nc.tensor.matmul(vps[:], wblk_v[:, 2 * k2:2 * k2 + 2, :],
                                 x2T[:, 2 * k2:2 * k2 + 2, ts(vsc, NCS)],
                                 start=(k2 == 0), stop=(k2 == KD2 - 1),
                                 perf_mode=DR)
                if k2 == KD2 - 1:
                    nc.vector.tensor_scalar(v[:, ts(vsc, NCS)], vps[:], V_SCALE,
                                            bin_sb[:, mo_v:mo_v + 1], op0=OP.mult, op1=OP.add)
            # v^T via PE transpose (bf16 -> fp8 on the copy out)
            vT = p2.tile([P, ST, P], dt.float8e4, tag="vT", name=f"vT_{h}")
            for tt in range(ST):
                pt = p2c.tile([P, P], dt.bfloat16, tag="vtp", name=f"vtp_{h}_{tt}")
                nc.tensor.transpose(pt[:], v[:, ts(tt, P)], ident[:])
                nc.vector.tensor_copy(vT[:, tt, :], pt[:])
            # denominator partial: tree-add the 8 expT t-tiles (DVE + GpSimd)
            t01 = p2.tile([P, S], dt.bfloat16, tag="t01", name=f"t01_{h}")
            nc.vector.tensor_tensor(t01[:], expT[:, 0, :], expT[:, 1, :], OP.add)
            t23 = p2.tile([P, S], dt.bfloat16, tag="t23", name=f"t23_{h}")
            nc.gpsimd.tensor_tensor(t23[:], expT[:, 2, :], expT[:, 3, :], OP.add)
            t45 = p2.tile([P, S], dt.bfloat16, tag="t45", name=f"t45_{h}")
            nc.vector.tensor_tensor(t45[:], expT[:, 4, :], expT[:, 5, :], OP.add)
            t67 = p2.tile([P, S], dt.bfloat16, tag="t67", name=f"t67_{h}")
            nc.gpsimd.tensor_tensor(t67[:], expT[:, 6, :], expT[:, 7, :], OP.add)
            ta = p2.tile([P, S], dt.bfloat16, tag="ta", name=f"ta_{h}")
            nc.vector.tensor_tensor(ta[:], t01[:], t23[:], OP.add)
            tb = p2.tile([P, S], dt.bfloat16, tag="tb", name=f"tb_{h}")
            nc.gpsimd.tensor_tensor(tb[:], t45[:], t67[:], OP.add)
            sum8 = p2.tile([P, S], dt.bfloat16, tag="sum8", name=f"sum8_{h}")
            nc.vector.tensor_tensor(sum8[:], ta[:], tb[:], OP.add)
            prev = (h, expT, vT, sum8)
        attn_tail(*prev)
        p2c.release()
        p2b.release()
        p2a.release()
        p2.release()
        p2w.release()
        xp.release()  # x2T dead

        # ===== phase 3: out_proj (fp8 DR) + residual + rmsnorm2 + quant =====
        # Two mt-groups: group 1's matmuls overlap group 0's rmsnorm/quant
        # chain, and ff1's first s-chunk needs exactly group 0's x2qT slices.
        qp = tc.alloc_tile_pool(name="x2qT_pool", bufs=1)
        x2qT = qp.tile([P, KD, S], dt.bfloat16)
        c1_bc = qp.tile([P, S], dt.float32)

        p3c = tc.alloc_tile_pool(name="p3c", bufs=1)
        p3q = tc.alloc_tile_pool(name="p3q", bufs=2)
        p3 = tc.alloc_tile_pool(name="p3", bufs=3)
        ps34 = tc.alloc_tile_pool(name="ps34", bufs=1, space="PSUM")
        n2w_sb = p3c.tile([P, D], dt.float32)
        nc.sync.dma_start(out=n2w_sb[:], in_=n2w_d.ap())
        b2_sb = p3c.tile([P, D], dt.float32)
        nc.sync.dma_start(out=b2_sb[:], in_=b2_d.ap())
        c1row = p3c.tile([1, S], dt.float32)

        def outproj_group(g):
            mts = range(g * GT, (g + 1) * GT)
            for no in range(NOD):
                psy = [ps34.tile([P, NCD], dt.float32, tag="acc", bufs=4,
                                 name=f"psy3_{g}_{no}_{i}") for i in range(GT)]
                for k2 in range(KD2):
                    wop = p3.tile([P, 2, NCD], dt.float8e4, tag="wop", bufs=6)
                    nc.sync.dma_start(out=wop[:], in_=wo_d.ap()[no, k2])
                    for i, mt in enumerate(mts):
                        nc.tensor.matmul(psy[i][:], oT_all[:, 2 * k2:2 * k2 + 2, ts(mt, P)],
                                         wop[:], start=(k2 == 0), stop=(k2 == KD2 - 1),
                                         perf_mode=DR)
                for i, mt in enumerate(mts):
                    ch = ts(no, NCD)
                    # x += o (descaled); single DVE op keeps the PSUM bank hot
                    nc.vector.scalar_tensor_tensor(xg[mt][:, ch], psy[i][:], OUT_SCALE,
                                                   xg[mt][:, ch], op0=OP.mult, op1=OP.add)
                    xo = p3.tile([P, NCD], dt.float32, tag="xo", bufs=3, name=f"xo_{no}_{mt}")
                    nc.vector.tensor_tensor(xo[:], xg[mt][:, ch], b2_sb[:, ch], OP.add)
                    nc.sync.dma_start(out=xb2_d.ap()[mt][:, ch], in_=xo[:])

        def quant_compute(mt):
            # rmsnorm2 + per-token absmax + int8 round (ACT/DVE only, no PE)
            xt = xg[mt]
            sq = p3q.tile([P, D], dt.float32, tag="scr", bufs=2, name=f"sq3_{mt}")
            ss = p3q.tile([P, 1], dt.float32, tag="ss", bufs=4)
            nc.scalar.activation(sq[:], xt[:], AF.Square, accum_out=ss[:])
            ms = p3q.tile([P, 1], dt.float32, tag="ms", bufs=4)
            nc.vector.tensor_scalar(ms[:], ss[:], 1.0 / D, EPS, op0=OP.mult, op1=OP.add)
            rt = p3q.tile([P, 1], dt.float32, tag="rt", bufs=4)
            nc.scalar.activation(rt[:], ms[:], AF.Sqrt)
            rs = p3q.tile([P, 1], dt.float32, tag="rs", bufs=4)
            nc.vector.reciprocal(rs[:], rt[:])
            x2 = p3q.tile([P, D], dt.float32, tag="x2", bufs=2)
            nc.vector.scalar_tensor_tensor(x2[:], xt[:], rs[:], n2w_sb[:],
                                           op0=OP.mult, op1=OP.mult)
            mx = p3q.tile([P, 1], dt.float32, tag="mx", bufs=4)
            nc.vector.tensor_reduce(mx[:], x2[:], axis=mybir.AxisListType.X,
                                    op=OP.max, apply_absolute_value=True)
            mcl = p3q.tile([P, 1], dt.float32, tag="mcl", bufs=4)
            nc.vector.tensor_scalar(mcl[:], mx[:], 1e-5, None, op0=OP.max)
            rc = p3q.tile([P, 1], dt.float32, tag="rc", bufs=4)
            nc.vector.reciprocal(rc[:], mcl[:])
            sc1 = p3q.tile([P, 1], dt.float32, tag="sc1", bufs=4)
            nc.vector.tensor_scalar(sc1[:], rc[:], 127.0, None, op0=OP.mult)
            c1c = p3q.tile([P, 1], dt.float32, tag="c1c", bufs=4)
            nc.vector.tensor_scalar(c1c[:], mcl[:], ws1 / 127.0, None, op0=OP.mult)
            nc.sync.dma_start(out=c1row[0:1, ts(mt, P)], in_=c1c[:])
            t1 = p3q.tile([P, D], dt.float32, tag="scr", bufs=2, name=f"t1_{mt}")
            nc.vector.tensor_scalar(t1[:], x2[:], sc1[:], MAGIC, op0=OP.mult, op1=OP.add)
            xq = p3q.tile([P, D], dt.bfloat16, tag="xq", bufs=4)
            nc.vector.tensor_scalar(xq[:], t1[:], MAGIC, None, op0=OP.subtract)
            return xq

        def quant_transpose(mt, xq):
            for ko in range(KD):
                pt = ps34.tile([P, P], dt.bfloat16, tag="tp", bufs=2)
                nc.tensor.transpose(pt[:], xq[:, ts(ko, P)], ident[:])
                nc.vector.tensor_copy(x2qT[:, ko, ts(mt, P)], pt[:])

        def c1_broadcast(g):
            pb = ps34.tile([P, NCS], dt.float32, tag="pb", bufs=2, name=f"pb3_{g}")
            nc.tensor.matmul(pb[:], ones_1f[:], c1row[:, ts(g, NCS)],
                             start=True, stop=True)
            nc.vector.tensor_copy(c1_bc[:, ts(g, NCS)], pb[:])

        # pipeline: group 1's matmuls cover group 0's quant chain; ff1's first
        # s-chunk (which needs only group 0) covers group 1's quant chain
        outproj_group(0)
        xqs0 = [quant_compute(mt) for mt in range(GT)]
        outproj_group(1)
        for mt in range(GT):
            quant_transpose(mt, xqs0[mt])
        c1_broadcast(0)
        xqs1 = [quant_compute(mt) for mt in range(GT, ST)]
        p3x.release()   # xg dead
        op_.release()   # oT_all dead

        # ================= phase 4: ff1 + snake + h2 spill + absmax =================
        p4 = tc.alloc_tile_pool(name="p4", bufs=3)
        p4m = tc.alloc_tile_pool(name="p4m", bufs=1)
        M_acc = p4m.tile([P, S], dt.float32)
        nc.any.memset(M_acc[:], 0.0)

        def ff1_block(sc, mo):
            wblk = p4.tile([P, KD, P], dt.bfloat16, tag="wblk")
            nc.sync.dma_start(out=wblk[:], in_=w1_d.ap()[mo])
            ph = ps34.tile([P, NCS], dt.float32, tag="acc", bufs=4, name=f"ph_{sc}_{mo}")
            for ko in range(KD):
                nc.tensor.matmul(ph[:], wblk[:, ko, :], x2qT[:, ko, ts(sc, NCS)],
                                 start=(ko == 0), stop=(ko == KD - 1))
            ch = ts(sc, NCS)
            t_ = p4.tile([P, NCS], dt.float32, tag="t_", name=f"t_{sc}_{mo}")
            nc.vector.tensor_tensor(t_[:], ph[:], c1_bc[:, ch], OP.mult)
            s_ = p4.tile([P, NCS], dt.float32, tag="s_", name=f"s_{sc}_{mo}")
            nc.scalar.activation(s_[:], t_[:], AF.Sin,
                                 scale=alpha_sb[:, mo:mo + 1], bias=ab1_sb[:, mo:mo + 1])
            h_ = p4.tile([P, NCS], dt.float32, tag="h_", name=f"h_{sc}_{mo}")
            nc.scalar.activation(h_[:], t_[:], AF.Identity, bias=b1_sb[:, mo:mo + 1])
            sq_ = p4.tile([P, NCS], dt.float32, tag="sq_", name=f"sq_{sc}_{mo}")
            nc.scalar.activation(sq_[:], s_[:], AF.Square)
            h2_ = p4.tile([P, NCS], dt.bfloat16, tag="h2_", name=f"h2_{sc}_{mo}")
            nc.vector.scalar_tensor_tensor(h2_[:], sq_[:], gam_sb[:, mo:mo + 1], h_[:],
                                           op0=OP.mult, op1=OP.add)
            nc.sync.dma_start(out=h2_d.ap()[mo][:, ch], in_=h2_[:])
            am_ = p4.tile([P, NCS], dt.float32, tag="am_", name=f"am_{sc}_{mo}")
            nc.scalar.activation(am_[:], h2_[:], AF.Abs)
            nc.vector.tensor_tensor(M_acc[:, ch], M_acc[:, ch], am_[:], OP.max)

        FF1_PRE = 8   # sc0 blocks emitted before group 1's transposes
        for mo in range(FF1_PRE):
            ff1_block(0, mo)
        for i, mt in enumerate(range(GT, ST)):
            quant_transpose(mt, xqs1[i])
        c1_broadcast(1)
        for mo in range(FF1_PRE, KF):
            ff1_block(0, mo)
        for mo in range(KF):
            ff1_block(1, mo)

        # cross-partition absmax via PE transpose + free-axis reduce
        m_tok = p4m.tile([P, ST], dt.float32)
        for c in range(ST):
            pmt = ps34.tile([P, P], dt.float32, tag="tp", bufs=2, name=f"pmt_{c}")
            nc.tensor.transpose(pmt[:], M_acc[:, ts(c, P)], identf[:])
            nc.vector.tensor_reduce(m_tok[:, c:c + 1], pmt[:], axis=mybir.AxisListType.X, op=OP.max)
        mcl2 = p4m.tile([P, ST], dt.float32)
        nc.vector.tensor_scalar(mcl2[:], m_tok[:], 1e-5, None, op0=OP.max)
        rc2 = p4m.tile([P, ST], dt.float32)
        nc.vector.reciprocal(rc2[:], mcl2[:])
        sc2_tok = p4m.tile([P, ST], dt.float32)
        nc.vector.tensor_scalar(sc2_tok[:], rc2[:], 127.0, None, op0=OP.mult)
        nc.vector.tensor_scalar(c2_tok[:], mcl2[:], ws2 / 127.0, None, op0=OP.mult)
        sc2row = p4m.tile([1, S], dt.float32)
        for mt in range(ST):
            nc.sync.dma_start(out=sc2row[0:1, ts(mt, P)], in_=sc2_tok[:, mt:mt + 1])
        for sc in range(NOS):
            pb = ps34.tile([P, NCS], dt.float32, tag="pb", bufs=2, name=f"pb4_{sc}")
            nc.tensor.matmul(pb[:], ones_1f[:], sc2row[:, ts(sc, NCS)], start=True, stop=True)
            nc.vector.tensor_copy(sc2_bc[:, ts(sc, NCS)], pb[:])
        p4m.release()
        p4.release()
        ps34.release()
        p3.release()
        p3q.release()
        p3c.release()
        qp.release()  # x2qT, c1_bc dead

        # ===== phase 5+6: ff2, with h2 quantization fused into the first no pass =====
        q2p = tc.alloc_tile_pool(name="xq2_pool", bufs=1)
        xq2 = q2p.tile([P, KF, S], dt.bfloat16)
        p6 = tc.alloc_tile_pool(name="p6", bufs=3)
        p6ps = tc.alloc_tile_pool(name="p6ps", bufs=1, space="PSUM")
        for no in range(NOD):
            psy = [p6ps.tile([P, NCD], dt.float32, tag=f"y{mt}", name=f"psy6_{no}_{mt}")
                   for mt in range(ST)]
            xchs = []
            for mt in range(ST):  # prefetch residual chunks
                xch = p6.tile([P, NCD], dt.float32, tag="xch", bufs=ST,
                              name=f"xch6_{no}_{mt}")
                nc.sync.dma_start(out=xch[:], in_=xb2_d.ap()[mt][:, ts(no, NCD)])
                xchs.append(xch)
            for ko in range(KF):
                if no == 0:
                    # quantize h2[ko] -> exact ints in bf16, just ahead of first use
                    h2t = p6.tile([P, S], dt.bfloat16, tag="h2t", bufs=6)
                    nc.sync.dma_start(out=h2t[:], in_=h2_d.ap()[ko])
                    m1 = p6.tile([P, S], dt.float32, tag="m1", bufs=2)
                    nc.vector.tensor_tensor(m1[:], h2t[:], sc2_bc[:], OP.mult)
                    nc.vector.tensor_scalar(xq2[:, ko, :], m1[:], MAGIC, MAGIC,
                                            op0=OP.add, op1=OP.subtract)
                wch = p6.tile([P, NCD], dt.bfloat16, tag="wch", bufs=4)
                nc.sync.dma_start(out=wch[:], in_=w2_d.ap()[no, ko])
                for mt in range(ST):
                    nc.tensor.matmul(psy[mt][:], xq2[:, ko, ts(mt, P)], wch[:],
                                     start=(ko == 0), stop=(ko == KF - 1))
            for mt in range(ST):
                # single-op evict: out = psum * c2[token] + (x + b2)
                oe = p6.tile([P, NCD], dt.float32, tag="oe", bufs=4, name=f"oe_{no}_{mt}")
                nc.vector.scalar_tensor_tensor(oe[:], psy[mt][:], c2_tok[:, mt:mt + 1],
                                               xchs[mt][:], op0=OP.mult, op1=OP.add)
                nc.sync.dma_start(out=out_d.ap()[ts(mt, P), ts(no, NCD)], in_=oe[:])
        p6ps.release()
        p6.release()
        q2p.release()
        cp.release()
    return nc


# ---------------------------------------------------------------- driver

def _get_compiled(key, S, D, H, DFF, ws1, ws2):
    if key in _CACHE:
        return _CACHE[key]
    from concourse import bacc

    nc = bacc.Bacc("TRN2", target_bir_lowering=False, debug=False, num_devices=NCORES)
    build_program(nc, S=S, D=D, H=H, DFF=DFF, ws1=ws1, ws2=ws2)
    nc.compile()
    _CACHE[key] = nc
    return nc


def make_in_maps(inputs):
    src = np.asarray(inputs["src"], dtype=np.float32)
    B, S, D = src.shape
    H = H_FULL
    DFF = inputs["ff1_w"].shape[0]
    arrays, ws1, ws2 = _prep_arrays(inputs, S, D, H, DFF)
    srcb = src + np.asarray(inputs["out_proj_b"], dtype=np.float32)[None, None, :]
    in_maps = []
    for c in range(NCORES):
        m = dict(arrays)
        m["src"] = np.ascontiguousarray(src[c])
        m["srcb"] = np.ascontiguousarray(srcb[c])
        in_maps.append(m)
    return in_maps, (S, D, H, DFF, ws1, ws2)


def kernel(**inputs):
    from concourse.bass_utils import run_bass_kernel_spmd

    in_maps, (S, D, H, DFF, ws1, ws2) = make_in_maps(inputs)
    assert np.asarray(inputs["src"]).shape[0] == NCORES
    nc = _get_compiled(("full", S, D, H, DFF, ws1, ws2), S, D, H, DFF, ws1, ws2)
    res = run_bass_kernel_spmd(nc, in_maps, core_ids=list(range(NCORES)))
    out = np.stack([res.results[c]["out"] for c in range(NCORES)], axis=0)
    return out.astype(np.float32)


# revision 34
# speedup vs baseline: 1.0703x; 1.0225x over previous
"""BitTransformerEncoderLayer on 8 TRN2 NeuronCores.

Strategy: pure data parallelism over batch (B=8 == n_cores); no collectives.
v2: fp8e4 DoubleRow matmuls (2x PE rate) for in_proj / out_proj / attn@v /
softmax-denominator; scores and the BitLinear FFN matmuls stay bf16 (the FFN
runs exact integer arithmetic in bf16 — fp8 would round ints > 16 and blow the
error budget). h2 spills to DRAM as bf16. Softmax denominators via DVE tree +
one ones-matmul; reciprocal_approx_fast for 1/denom.

kernel(**inputs) takes the FULL unsharded inputs and returns the FULL output.
"""

import numpy as np

P = 128
EPS = 1e-8
MAGIC = 12582912.0  # 1.5 * 2**23: fp32 add/sub rounds to nearest-even integer
NCORES = 8
EXP_SHIFT = 2.0     # exp(scores - 2): max exp ~125 < 240 (TRN e4m3 max)
WSCALE = 64.0       # fp8 weight pre-scale for in_proj/out_proj
VSCALE = 16.0       # fp8 v pre-scale
OSCALE = 32.0       # fp8 o (attn out) pre-scale

# Problem dims (hardcoded per spec)
B_FULL, S_FULL, D_FULL, H_FULL, DFF_FULL = 8, 1024, 2048, 16, 8192

_CACHE = {}


# ---------------------------------------------------------------- host prep

def _quant_w(w):
    scale = np.maximum(np.mean(np.abs(w), dtype=np.float32), np.float32(1e-5))
    q = np.clip(np.round(w / scale), -1.0, 1.0).astype(np.float32)
    return q, float(scale)


def _lhsT_blocks(w):
    """w [M, K] -> [M/P, P(k), K/P, P(m)]; [mo, :, ko, :] = w-block(mo, ko).T"""
    M, K = w.shape
    t = w.reshape(M // P, P, K // P, P)  # [mo, pm, ko, pk]
    return np.ascontiguousarray(t.transpose(0, 3, 2, 1))


def _rhs_chunks(w, nch):
    """w [N, K] -> [N/nch, K/P, P, nch]; [no, ko, p, j] = w[no*nch+j, ko*P+p]"""
    N, K = w.shape
    t = w.reshape(N // nch, nch, K // P, P)  # [no, j, ko, p]
    return np.ascontiguousarray(t.transpose(0, 2, 3, 1))


def _rhs_pair_chunks(w, nch):
    """w [N, K] -> [N/nch, K/2P, P, 2, nch]; [no, k2, p, i, j] = w[no*nch+j, (2*k2+i)*P+p]"""
    N, K = w.shape
    t = w.reshape(N // nch, nch, K // (2 * P), 2, P)  # [no, j, k2, i, p]
    return np.ascontiguousarray(t.transpose(0, 2, 4, 3, 1))


def _per_part(v):
    """[M] -> [P, M/P]; out[p, mo] = v[mo*P + p]"""
    return np.ascontiguousarray(v.reshape(-1, P).T)


def _bcast_row(v):
    return np.ascontiguousarray(np.broadcast_to(v[None, :], (P, v.shape[0])))


def _prep_arrays(inputs, S, D, H, DFF):
    import ml_dtypes

    bf16 = ml_dtypes.bfloat16
    f8 = ml_dtypes.float8_e4m3fn
    f32 = np.float32
    g = lambda k: np.asarray(inputs[k], dtype=f32)

    w1q, ws1 = _quant_w(g("ff1_w"))   # [DFF, D]
    w2q, ws2 = _quant_w(g("ff2_w"))   # [D, DFF]
    ncd = min(512, D)

    bias_in = _per_part(g("in_proj_b")).astype(f32)  # [P, 3D/P]
    bias_in[:, 2 * (D // P):] *= VSCALE  # v eviction is pre-scaled by VSCALE

    arrays = {
        "w_in_blk": _lhsT_blocks(g("in_proj_w") * WSCALE).astype(f8),  # [3D/P, P, D/P, P]
        "wo_pair": _rhs_pair_chunks(g("out_proj_w") * WSCALE, ncd).astype(f8),
        "w1_blk": _lhsT_blocks(w1q).astype(bf16),                # [DFF/P, P, D/P, P]
        "w2_chunk": _rhs_chunks(w2q, ncd).astype(bf16),          # [D/ncd, DFF/P, P, ncd]
        "bias_in": bias_in,                                      # [P, 3D/P]
        "b1_t": _per_part(g("ff1_b")).astype(f32),               # [P, DFF/P]
        "alpha_t": _per_part(g("alpha")).astype(f32),
        "ab1_t": _per_part((g("alpha") * g("ff1_b")).astype(f32)),
        "gamma_t": _per_part((1.0 / (g("beta") + np.float32(1e-9))).astype(f32)),
        "n1w_bc": _bcast_row(g("norm1_w")).astype(f32),          # [P, D]
        "n2w_bc": _bcast_row(g("norm2_w")).astype(f32),
        "b2_bc": _bcast_row(g("ff2_b")).astype(f32),
    }
    return arrays, ws1, ws2


# ---------------------------------------------------------------- device program

def build_program(nc, *, S, D, H, DFF, ws1, ws2):
    import concourse.mybir as mybir
    import concourse.tile as tile
    from concourse.bass import ts
    from concourse.masks import make_identity

    dt = mybir.dt
    AF = mybir.ActivationFunctionType
    OP = mybir.AluOpType
    DR = mybir.MatmulPerfMode.DoubleRow

    DH = D // H
    assert DH == P, "layout assumes head dim == 128"
    ST = S // P           # token tiles
    KD = D // P           # D contraction tiles
    KD2 = KD // 2         # DoubleRow k-pairs over D
    KF = DFF // P         # DFF contraction tiles / ff1 out tiles
    NCD = min(512, D)     # fo chunk for out_proj/ff2 (psum-bank sized)
    NOD = D // NCD
    NCS = min(512, S)     # s chunk
    NOS = S // NCS
    GT = ST // 2          # phase-3 mt-group size
    inv_sqrt_dh = float(1.0 / np.sqrt(DH))
    OUT_SCALE = float(1.0 / (OSCALE * WSCALE))   # out_proj psum descale
    QK_SCALE = float(1.0 / WSCALE)               # q/k eviction descale
    V_SCALE = float(VSCALE / WSCALE)             # v eviction: VSCALE * (1/WSCALE)

    # ---- DRAM I/O ----
    src_d = nc.dram_tensor("src", [S, D], dt.float32, kind="ExternalInput")
    srcb_d = nc.dram_tensor("srcb", [S, D], dt.float32, kind="ExternalInput")  # src + out_proj_b
    w_in_d = nc.dram_tensor("w_in_blk", [3 * KD, P, KD, P], dt.float8e4, kind="ExternalInput")
    wo_d = nc.dram_tensor("wo_pair", [NOD, KD2, P, 2, NCD], dt.float8e4, kind="ExternalInput")
    w1_d = nc.dram_tensor("w1_blk", [KF, P, KD, P], dt.bfloat16, kind="ExternalInput")
    w2_d = nc.dram_tensor("w2_chunk", [NOD, KF, P, NCD], dt.bfloat16, kind="ExternalInput")
    bin_d = nc.dram_tensor("bias_in", [P, 3 * KD], dt.float32, kind="ExternalInput")
    b1_d = nc.dram_tensor("b1_t", [P, KF], dt.float32, kind="ExternalInput")
    alpha_d = nc.dram_tensor("alpha_t", [P, KF], dt.float32, kind="ExternalInput")
    ab1_d = nc.dram_tensor("ab1_t", [P, KF], dt.float32, kind="ExternalInput")
    gam_d = nc.dram_tensor("gamma_t", [P, KF], dt.float32, kind="ExternalInput")
    n1w_d = nc.dram_tensor("n1w_bc", [P, D], dt.float32, kind="ExternalInput")
    n2w_d = nc.dram_tensor("n2w_bc", [P, D], dt.float32, kind="ExternalInput")
    b2_d = nc.dram_tensor("b2_bc", [P, D], dt.float32, kind="ExternalInput")
    out_d = nc.dram_tensor("out", [S, D], dt.float32, kind="ExternalOutput")
    # internal DRAM spills
    xb2_d = nc.dram_tensor("xb2_spill", [ST, P, D], dt.float32)  # x + b2
    h2_d = nc.dram_tensor("h2_spill", [KF, P, S], dt.bfloat16)

    with tile.TileContext(nc) as tc:
        # ---------- persistent constants (whole kernel) ----------
        cp = tc.alloc_tile_pool(name="consts", bufs=1)
        ident = cp.tile([P, P], dt.bfloat16)
        make_identity(nc, ident)
        identf = cp.tile([P, P], dt.float32)
        make_identity(nc, identf)
        half_kb = cp.tile([P, 1], dt.bfloat16)
        nc.any.memset(half_kb[:], 0.5)   # folds rb = 2/denom
        ones_1 = cp.tile([1, P], dt.bfloat16)
        nc.any.memset(ones_1[:], 1.0)
        ones_1f = cp.tile([1, P], dt.float32)
        nc.any.memset(ones_1f[:], 1.0)
        eshift = cp.tile([P, 1], dt.float32)
        nc.any.memset(eshift[:], -EXP_SHIFT)
        bin_sb = cp.tile([P, 3 * KD], dt.float32)
        b1_sb = cp.tile([P, KF], dt.float32)
        alpha_sb = cp.tile([P, KF], dt.float32)
        ab1_sb = cp.tile([P, KF], dt.float32)
        gam_sb = cp.tile([P, KF], dt.float32)
        c2_tok = cp.tile([P, ST], dt.float32)   # filled in phase 4
        sc2_bc = cp.tile([P, S], dt.float32)    # filled in phase 4
        wpre0 = cp.tile([P, KD, P], dt.float8e4, name="wpre_0")

        # ================= phase 1: rmsnorm1 + transpose (fp8) =================
        xp = tc.alloc_tile_pool(name="x2T_pool", bufs=1)
        x2T = xp.tile([P, KD, S], dt.float8e4)

        p1 = tc.alloc_tile_pool(name="p1", bufs=2)
        p1c = tc.alloc_tile_pool(name="p1c", bufs=1)
        p1ps = tc.alloc_tile_pool(name="p1ps", bufs=4, space="PSUM")
        # DMA issue order: n1w + src tiles first (they gate the critical
        # chain), head-0 weights next, small phase-2/4 consts last
        n1w_sb = p1c.tile([P, D], dt.float32)
        nc.sync.dma_start(out=n1w_sb[:], in_=n1w_d.ap())
        for mt in range(ST):
            xt = p1.tile([P, D], dt.float32, tag="xt", bufs=8)
            nc.sync.dma_start(out=xt[:], in_=src_d.ap()[ts(mt, P), :])
            sq = p1.tile([P, D], dt.float32, tag="sq", bufs=3)
            ss = p1.tile([P, 1], dt.float32, tag="ss")
            nc.scalar.activation(sq[:], xt[:], AF.Square, accum_out=ss[:])
            ms = p1.tile([P, 1], dt.float32, tag="ms")
            nc.vector.tensor_scalar(ms[:], ss[:], 1.0 / D, EPS, op0=OP.mult, op1=OP.add)
            rt = p1.tile([P, 1], dt.float32, tag="rt")
            nc.scalar.activation(rt[:], ms[:], AF.Sqrt)
            rs = p1.tile([P, 1], dt.float32, tag="rs")
            nc.vector.reciprocal(rs[:], rt[:])
            x2 = p1.tile([P, D], dt.bfloat16, tag="x2", bufs=3)
            nc.vector.scalar_tensor_tensor(x2[:], xt[:], rs[:], n1w_sb[:], op0=OP.mult, op1=OP.mult)
            for ko in range(KD):
                pt = p1ps.tile([P, P], dt.bfloat16, tag="tp")
                nc.tensor.transpose(pt[:], x2[:, ts(ko, P)], ident[:])
                nc.vector.tensor_copy(x2T[:, ko, ts(mt, P)], pt[:])
            if mt == 0:  # src mt0..7 + n1w issued; now the head-0 weights
                nc.sync.dma_start(out=wpre0[:], in_=w_in_d.ap()[0])
        nc.sync.dma_start(out=bin_sb[:], in_=bin_d.ap())
        nc.sync.dma_start(out=b1_sb[:], in_=b1_d.ap())
        nc.sync.dma_start(out=alpha_sb[:], in_=alpha_d.ap())
        nc.sync.dma_start(out=ab1_sb[:], in_=ab1_d.ap())
        nc.sync.dma_start(out=gam_sb[:], in_=gam_d.ap())
        p1ps.release()
        p1c.release()
        p1.release()

        # ================= phase 2: fused in_proj (fp8 DR) + attention =================
        op_ = tc.alloc_tile_pool(name="oT_pool", bufs=1, side="right")
        oT_all = op_.tile([P, KD, S], dt.float8e4)
        # residual tiles for phase 3, seeded during phase 2 (DMA slack)
        p3x = tc.alloc_tile_pool(name="p3x", bufs=1, side="right")
        xg = [p3x.tile([P, D], dt.float32, tag=f"xg{i}", name=f"xg_{i}")
              for i in range(ST)]

        p2w = tc.alloc_tile_pool(name="p2w", bufs=2)
        p2 = tc.alloc_tile_pool(name="p2", bufs=2)
        p2a = tc.alloc_tile_pool(name="p2a", bufs=2, space="PSUM")
        p2b = tc.alloc_tile_pool(name="p2b", bufs=2, space="PSUM")
        p2c = tc.alloc_tile_pool(name="p2c", bufs=1, space="PSUM")

        def attn_tail(h, expT, vT, sum8):
            # softmax denominator: partition-reduce sum8 on PE, broadcast,
            # fast-reciprocal; then o^T = (sum_t v16^T @ exp^T) * (2/denom).
            # Emitted one head late so the PE never waits on ACT's exp.
            rb = p2.tile([P, S], dt.float32, tag="rb", name=f"rb_{h}")
            for sc in range(NOS):
                psr = p2c.tile([1, NCS], dt.float32, tag="den", name=f"psr_{h}_{sc}")
                nc.tensor.matmul(psr[:], half_kb[:], sum8[:, ts(sc, NCS)],
                                 start=True, stop=True)
                srow = p2.tile([1, NCS], dt.bfloat16, tag="srow", name=f"srow_{h}_{sc}")
                nc.scalar.activation(srow[:], psr[:], AF.Copy)
                psb = p2c.tile([P, NCS], dt.float32, tag="den", name=f"psb_{h}_{sc}")
                nc.tensor.matmul(psb[:], ones_1[:], srow[:], start=True, stop=True)
                nc.vector.reciprocal_approx_fast(rb[:, ts(sc, NCS)], psb[:])
            for sc in range(NOS):
                po = p2a.tile([P, NCS], dt.float32, tag="po", bufs=2, name=f"po_{h}_{sc}")
                for t2 in range(ST // 2):
                    nc.tensor.matmul(po[:], vT[:, 2 * t2:2 * t2 + 2, :],
                                     expT[:, 2 * t2:2 * t2 + 2, ts(sc, NCS)],
                                     start=(t2 == 0), stop=(t2 == ST // 2 - 1),
                                     perf_mode=DR)
                nc.vector.tensor_tensor(oT_all[:, h, ts(sc, NCS)], po[:], rb[:, ts(sc, NCS)],
                                        OP.mult)

        def qkv_block(h, j, mo, out_dtype, scale):
            # one projection (q/k/v) for head h: 2 s-chunks x 8 DR matmuls,
            # evicted on DVE (scale + bias) to keep ACT free for exp
            if h == 0 and j == 0:
                wblk = wpre0
            else:
                wblk = p2w.tile([P, KD, P], dt.float8e4, tag="wblk", bufs=4)
                nc.sync.dma_start(out=wblk[:], in_=w_in_d.ap()[mo])
            dest = p2.tile([P, S], out_dtype, tag=f"qkv{j}", name=f"qkv{j}_{h}")
            for sc in range(NOS):
                ps = p2a.tile([P, NCS], dt.float32, tag="mmps", bufs=2,
                              name=f"qkvps_{h}_{j}_{sc}")
                for k2 in range(KD2):
                    nc.tensor.matmul(ps[:], wblk[:, 2 * k2:2 * k2 + 2, :],
                                     x2T[:, 2 * k2:2 * k2 + 2, ts(sc, NCS)],
                                     start=(k2 == 0), stop=(k2 == KD2 - 1),
                                     perf_mode=DR)
                nc.vector.tensor_scalar(dest[:, ts(sc, NCS)], ps[:], scale,
                                        bin_sb[:, mo:mo + 1], op0=OP.mult, op1=OP.add)
            return dest

        prev = None
        for h in range(H):
            if 4 <= h < 4 + ST:  # spread the 8 residual seeds across heads
                nc.sync.dma_start(out=xg[h - 4][:], in_=srcb_d.ap()[ts(h - 4, P), :])
            k = qkv_block(h, 1, KD + h, dt.bfloat16, QK_SCALE)
            q = qkv_block(h, 0, h, dt.bfloat16, QK_SCALE)
            # prev head's denominator + attn@v fill the PE while this head's
            # exp evictions (ACT) drain into the next head's DR stream
            if prev is not None:
                attn_tail(*prev)
            # v projection woven between the scores matmuls: the 450ns ACT exp
            # evictions pace the 2-bank scps rotation, so pure scores bursts
            # would stall the PE ~50% — the v DRs fill those slots
            mo_v = 2 * KD + h
            wblk_v = p2w.tile([P, KD, P], dt.float8e4, tag="wblk", bufs=4)
            nc.sync.dma_start(out=wblk_v[:], in_=w_in_d.ap()[mo_v])
            v = p2.tile([P, S], dt.bfloat16, tag="qkv2", name=f"qkv2_{h}")
            expT = p2.tile([P, ST, S], dt.float8e4, tag="expT", name=f"expT_{h}")
            vps = None
            for i in range(2 * ST):  # 16 weave slots
                tt, sc = divmod(i, NOS)
                ps = p2b.tile([P, NCS], dt.float32, tag="scps", name=f"scps_{h}_{tt}_{sc}")
                nc.tensor.matmul(ps[:], k[:, ts(tt, P)], q[:, ts(sc, NCS)],
                                 start=True, stop=True)
                nc.scalar.activation(expT[:, tt, ts(sc, NCS)], ps[:], AF.Exp,
                                     scale=inv_sqrt_dh, bias=eshift[:])
                vsc, k2 = divmod(i, KD2)
                if k2 == 0:
                    vps = p2a.tile([P, NCS], dt.float32, tag="mmps", bufs=2,
                                   name=f"qkvps_{h}_2_{vsc}")
                nc.tensor.matmul(vps[:], wblk_v[:, 2 * k2:2 * k2 + 2, :],
                                 x2T[:, 2 * k2:2 * k2 + 2, ts(vsc, NCS)],
                                 start=(k2 == 0), stop=(k2 == KD2 - 1),
                                 perf_mode=DR)
                if k2 == KD2 - 1:
                    nc.vector.tensor_scalar(v[:, ts(vsc, NCS)], vps[:], V_SCALE,
                                            bin_sb[:, mo_v:mo_v + 1], op0=OP.mult, op1=OP.add)
            # v^T via PE transpose (bf16 -> fp8 on the copy out)
            vT = p2.tile([P, ST, P], dt.float8e4, tag="vT", name=f"vT_{h}")
            for tt in range(ST):
                pt = p2c.tile([P, P], dt.bfloat16, tag="vtp", name=f"vtp_{h}_{tt}")
                nc.tensor.transpose(pt[:], v[:, ts(tt, P)], ident[:])
                nc.vector.tensor_copy(vT[:, tt, :], pt[:])
            # denominator partial: tree-add the 8 expT t-tiles (DVE)
            t01 = p2.tile([P, S], dt.bfloat16, tag="t01", name=f"t01_{h}")
            nc.vector.tensor_tensor(t01[:], expT[:, 0, :], expT[:, 1, :], OP.add)
            t23 = p2.tile([P, S], dt.bfloat16, tag="t23", name=f"t23_{h}")
            nc.vector.tensor_tensor(t23[:], expT[:, 2, :], expT[:, 3, :], OP.add)
            t45 = p2.tile([P, S], dt.bfloat16, tag="t45", name=f"t45_{h}")
            nc.vector.tensor_tensor(t45[:], expT[:, 4, :], expT[:, 5, :], OP.add)
            t67 = p2.tile([P, S], dt.bfloat16, tag="t67", name=f"t67_{h}")
            nc.vector.tensor_tensor(t67[:], expT[:, 6, :], expT[:, 7, :], OP.add)
            ta = p2.tile([P, S], dt.bfloat16, tag="ta", name=f"ta_{h}")
            nc.vector.tensor_tensor(ta[:], t01[:], t23[:], OP.add)
            tb = p2.tile([P, S], dt.bfloat16, tag="tb", name=f"tb_{h}")
            nc.vector.tensor_tensor(tb[:], t45[:], t67[:], OP.add)
            sum8 = p2.tile([P, S], dt.bfloat16, tag="sum8", name=f"sum8_{h}")
            nc.vector.tensor_tensor(sum8[:], ta[:], tb[:], OP.add)
            prev = (h, expT, vT, sum8)
        attn_tail(*prev)
        p2c.release()
        p2b.release()
        p2a.release()
        p2.release()
        p2w.release()
        xp.release()  # x2T dead

        # ===== phase 3: out_proj (fp8 DR) + residual + rmsnorm2 + quant =====
        # Two mt-groups: group 1's matmuls overlap group 0's rmsnorm/quant
        # chain, and ff1's first s-chunk needs exactly group 0's x2qT slices.
        qp = tc.alloc_tile_pool(name="x2qT_pool", bufs=1)
        x2qT = qp.tile([P, KD, S], dt.bfloat16)
        c1_bc = qp.tile([P, S], dt.float32)

        p3c = tc.alloc_tile_pool(name="p3c", bufs=1)
        p3q = tc.alloc_tile_pool(name="p3q", bufs=2)
        p3 = tc.alloc_tile_pool(name="p3", bufs=3)
        ps34 = tc.alloc_tile_pool(name="ps34", bufs=1, space="PSUM")
        n2w_sb = p3c.tile([P, D], dt.float32)
        nc.sync.dma_start(out=n2w_sb[:], in_=n2w_d.ap())
        b2_sb = p3c.tile([P, D], dt.float32)
        nc.sync.dma_start(out=b2_sb[:], in_=b2_d.ap())
        c1row = p3c.tile([1, S], dt.float32)

        def outproj_group(g):
            mts = range(g * GT, (g + 1) * GT)
            for no in range(NOD):
                psy = [ps34.tile([P, NCD], dt.float32, tag="acc", bufs=4,
                                 name=f"psy3_{g}_{no}_{i}") for i in range(GT)]
                for k2 in range(KD2):
                    wop = p3.tile([P, 2, NCD], dt.float8e4, tag="wop", bufs=6)
                    nc.sync.dma_start(out=wop[:], in_=wo_d.ap()[no, k2])
                    for i, mt in enumerate(mts):
                        nc.tensor.matmul(psy[i][:], oT_all[:, 2 * k2:2 * k2 + 2, ts(mt, P)],
                                         wop[:], start=(k2 == 0), stop=(k2 == KD2 - 1),
                                         perf_mode=DR)
                for i, mt in enumerate(mts):
                    ch = ts(no, NCD)
                    # x += o (descaled); single DVE op keeps the PSUM bank hot
                    nc.vector.scalar_tensor_tensor(xg[mt][:, ch], psy[i][:], OUT_SCALE,
                                                   xg[mt][:, ch], op0=OP.mult, op1=OP.add)
                    xo = p3.tile([P, NCD], dt.float32, tag="xo", bufs=3, name=f"xo_{no}_{mt}")
                    nc.vector.tensor_tensor(xo[:], xg[mt][:, ch], b2_sb[:, ch], OP.add)
                    nc.sync.dma_start(out=xb2_d.ap()[mt][:, ch], in_=xo[:])

        def quant_compute(mt):
            # rmsnorm2 + per-token absmax + int8 round (ACT/DVE only, no PE)
            xt = xg[mt]
            sq = p3q.tile([P, D], dt.float32, tag="scr", bufs=2, name=f"sq3_{mt}")
            ss = p3q.tile([P, 1], dt.float32, tag="ss", bufs=4)
            nc.scalar.activation(sq[:], xt[:], AF.Square, accum_out=ss[:])
            ms = p3q.tile([P, 1], dt.float32, tag="ms", bufs=4)
            nc.vector.tensor_scalar(ms[:], ss[:], 1.0 / D, EPS, op0=OP.mult, op1=OP.add)
            rt = p3q.tile([P, 1], dt.float32, tag="rt", bufs=4)
            nc.scalar.activation(rt[:], ms[:], AF.Sqrt)
            rs = p3q.tile([P, 1], dt.float32, tag="rs", bufs=4)
            nc.vector.reciprocal(rs[:], rt[:])
            x2 = p3q.tile([P, D], dt.float32, tag="x2", bufs=2)
            nc.vector.scalar_tensor_tensor(x2[:], xt[:], rs[:], n2w_sb[:],
                                           op0=OP.mult, op1=OP.mult)
            mx = p3q.tile([P, 1], dt.float32, tag="mx", bufs=4)
            nc.vector.tensor_reduce(mx[:], x2[:], axis=mybir.AxisListType.X,
                                    op=OP.max, apply_absolute_value=True)
            mcl = p3q.tile([P, 1], dt.float32, tag="mcl", bufs=4)
            nc.vector.tensor_scalar(mcl[:], mx[:], 1e-5, None, op0=OP.max)
            rc = p3q.tile([P, 1], dt.float32, tag="rc", bufs=4)
            nc.vector.reciprocal(rc[:], mcl[:])
            sc1 = p3q.tile([P, 1], dt.float32, tag="sc1", bufs=4)
            nc.vector.tensor_scalar(sc1[:], rc[:], 127.0, None, op0=OP.mult)
            c1c = p3q.tile([P, 1], dt.float32, tag="c1c", bufs=4)
            nc.vector.tensor_scalar(c1c[:], mcl[:], ws1 / 127.0, None, op0=OP.mult)
            nc.sync.dma_start(out=c1row[0:1, ts(mt, P)], in_=c1c[:])
            t1 = p3q.tile([P, D], dt.float32, tag="scr", bufs=2, name=f"t1_{mt}")
            nc.vector.tensor_scalar(t1[:], x2[:], sc1[:], MAGIC, op0=OP.mult, op1=OP.add)
            xq = p3q.tile([P, D], dt.bfloat16, tag="xq", bufs=4)
            nc.vector.tensor_scalar(xq[:], t1[:], MAGIC, None, op0=OP.subtract)
            return xq

        def quant_transpose(mt, xq):
            for ko in range(KD):
                pt = ps34.tile([P, P], dt.bfloat16, tag="tp", bufs=2)
                nc.tensor.transpose(pt[:], xq[:, ts(ko, P)], ident[:])
                nc.vector.tensor_copy(x2qT[:, ko, ts(mt, P)], pt[:])

        def c1_broadcast(g):
            pb = ps34.tile([P, NCS], dt.float32, tag="pb", bufs=2, name=f"pb3_{g}")
            nc.tensor.matmul(pb[:], ones_1f[:], c1row[:, ts(g, NCS)],
                             start=True, stop=True)
            nc.vector.tensor_copy(c1_bc[:, ts(g, NCS)], pb[:])

        # pipeline: group 1's matmuls cover group 0's quant chain; ff1's first
        # s-chunk (which needs only group 0) covers group 1's quant chain
        outproj_group(0)
        xqs0 = [quant_compute(mt) for mt in range(GT)]
        outproj_group(1)
        for mt in range(GT):
            quant_transpose(mt, xqs0[mt])
        c1_broadcast(0)
        xqs1 = [quant_compute(mt) for mt in range(GT, ST)]
        p3x.release()   # xg dead
        op_.release()   # oT_all dead

        # ================= phase 4: ff1 + snake + h2 spill + absmax =================
        p4 = tc.alloc_tile_pool(name="p4", bufs=3)
        p4m = tc.alloc_tile_pool(name="p4m", bufs=1)
        M_acc = p4m.tile([P, S], dt.float32)
        nc.any.memset(M_acc[:], 0.0)

        def ff1_block(sc, mo):
            wblk = p4.tile([P, KD, P], dt.bfloat16, tag="wblk")
            nc.sync.dma_start(out=wblk[:], in_=w1_d.ap()[mo])
            ph = ps34.tile([P, NCS], dt.float32, tag="acc", bufs=4, name=f"ph_{sc}_{mo}")
            for ko in range(KD):
                nc.tensor.matmul(ph[:], wblk[:, ko, :], x2qT[:, ko, ts(sc, NCS)],
                                 start=(ko == 0), stop=(ko == KD - 1))
            ch = ts(sc, NCS)
            t_ = p4.tile([P, NCS], dt.float32, tag="t_", name=f"t_{sc}_{mo}")
            nc.vector.tensor_tensor(t_[:], ph[:], c1_bc[:, ch], OP.mult)
            s_ = p4.tile([P, NCS], dt.float32, tag="s_", name=f"s_{sc}_{mo}")
            nc.scalar.activation(s_[:], t_[:], AF.Sin,
                                 scale=alpha_sb[:, mo:mo + 1], bias=ab1_sb[:, mo:mo + 1])
            h_ = p4.tile([P, NCS], dt.float32, tag="h_", name=f"h_{sc}_{mo}")
            nc.scalar.activation(h_[:], t_[:], AF.Identity, bias=b1_sb[:, mo:mo + 1])
            sq_ = p4.tile([P, NCS], dt.float32, tag="sq_", name=f"sq_{sc}_{mo}")
            nc.scalar.activation(sq_[:], s_[:], AF.Square)
            h2_ = p4.tile([P, NCS], dt.bfloat16, tag="h2_", name=f"h2_{sc}_{mo}")
            nc.vector.scalar_tensor_tensor(h2_[:], sq_[:], gam_sb[:, mo:mo + 1], h_[:],
                                           op0=OP.mult, op1=OP.add)
            nc.sync.dma_start(out=h2_d.ap()[mo][:, ch], in_=h2_[:])
            am_ = p4.tile([P, NCS], dt.float32, tag="am_", name=f"am_{sc}_{mo}")
            nc.scalar.activation(am_[:], h2_[:], AF.Abs)
            nc.vector.tensor_tensor(M_acc[:, ch], M_acc[:, ch], am_[:], OP.max)

        m_tok = p4m.tile([P, ST], dt.float32)
        mcl2 = p4m.tile([P, ST], dt.float32)
        rc2 = p4m.tile([P, ST], dt.float32)
        sc2_tok = p4m.tile([P, ST], dt.float32)
        sc2row = p4m.tile([1, S], dt.float32)

        def absmax_half(sc):
            # cross-partition absmax for this token half; overlaps the other
            # half's ff1 stream (PE cost: 4 small transposes + 1 broadcast)
            lo = sc * GT
            for c in range(lo, lo + GT):
                pmt = ps34.tile([P, P], dt.float32, tag="tp", bufs=2, name=f"pmt_{c}")
                nc.tensor.transpose(pmt[:], M_acc[:, ts(c, P)], identf[:])
                nc.vector.tensor_reduce(m_tok[:, c:c + 1], pmt[:],
                                        axis=mybir.AxisListType.X, op=OP.max)
            hs = slice(lo, lo + GT)
            nc.vector.tensor_scalar(mcl2[:, hs], m_tok[:, hs], 1e-5, None, op0=OP.max)
            nc.vector.reciprocal(rc2[:, hs], mcl2[:, hs])
            nc.vector.tensor_scalar(sc2_tok[:, hs], rc2[:, hs], 127.0, None, op0=OP.mult)
            nc.vector.tensor_scalar(c2_tok[:, hs], mcl2[:, hs], ws2 / 127.0, None, op0=OP.mult)
            for mt in range(lo, lo + GT):
                nc.sync.dma_start(out=sc2row[0:1, ts(mt, P)], in_=sc2_tok[:, mt:mt + 1])
            pb = ps34.tile([P, NCS], dt.float32, tag="pb", bufs=2, name=f"pb4_{sc}")
            nc.tensor.matmul(pb[:], ones_1f[:], sc2row[:, ts(sc, NCS)], start=True, stop=True)
            nc.vector.tensor_copy(sc2_bc[:, ts(sc, NCS)], pb[:])

        FF1_PRE = 8   # sc0 blocks emitted before group 1's transposes
        for mo in range(FF1_PRE):
            ff1_block(0, mo)
        for i, mt in enumerate(range(GT, ST)):
            quant_transpose(mt, xqs1[i])
        c1_broadcast(1)
        for mo in range(FF1_PRE, KF):
            ff1_block(0, mo)
        absmax_half(0)
        for mo in range(KF):
            ff1_block(1, mo)
        absmax_half(1)
        p4m.release()
        p4.release()
        ps34.release()
        p3.release()
        p3q.release()
        p3c.release()
        qp.release()  # x2qT, c1_bc dead

        # ===== phase 5+6: ff2, with h2 quantization fused into the first no pass =====
        q2p = tc.alloc_tile_pool(name="xq2_pool", bufs=1)
        xq2 = q2p.tile([P, KF, S], dt.bfloat16)
        p6 = tc.alloc_tile_pool(name="p6", bufs=3)
        p6ps = tc.alloc_tile_pool(name="p6ps", bufs=1, space="PSUM")
        for no in range(NOD):
            psy = [p6ps.tile([P, NCD], dt.float32, tag=f"y{mt}", name=f"psy6_{no}_{mt}")
                   for mt in range(ST)]
            xchs = [None] * ST
            for ko in range(KF):
                if 2 <= ko < 2 + ST:
                    # residual chunk loads woven in so they don't delay the
                    # weight/h2 DMAs at the no boundary
                    mt = ko - 2
                    xch = p6.tile([P, NCD], dt.float32, tag="xch", bufs=ST + 2,
                                  name=f"xch6_{no}_{mt}")
                    nc.sync.dma_start(out=xch[:], in_=xb2_d.ap()[mt][:, ts(no, NCD)])
                    xchs[mt] = xch
                if no == 0:
                    # quantize h2[ko] -> exact ints in bf16, just ahead of first use
                    h2t = p6.tile([P, S], dt.bfloat16, tag="h2t", bufs=6)
                    nc.sync.dma_start(out=h2t[:], in_=h2_d.ap()[ko])
                    m1 = p6.tile([P, S], dt.float32, tag="m1", bufs=2)
                    nc.vector.tensor_tensor(m1[:], h2t[:], sc2_bc[:], OP.mult)
                    nc.vector.tensor_scalar(xq2[:, ko, :], m1[:], MAGIC, MAGIC,
                                            op0=OP.add, op1=OP.subtract)
                wch = p6.tile([P, NCD], dt.bfloat16, tag="wch", bufs=4)
                nc.sync.dma_start(out=wch[:], in_=w2_d.ap()[no, ko])
                for mt in range(ST):
                    nc.tensor.matmul(psy[mt][:], xq2[:, ko, ts(mt, P)], wch[:],
                                     start=(ko == 0), stop=(ko == KF - 1))
            for mt in range(ST):
                # single-op evict: out = psum * c2[token] + (x + b2)
                oe = p6.tile([P, NCD], dt.float32, tag="oe", bufs=4, name=f"oe_{no}_{mt}")
                nc.vector.scalar_tensor_tensor(oe[:], psy[mt][:], c2_tok[:, mt:mt + 1],
                                               xchs[mt][:], op0=OP.mult, op1=OP.add)
                nc.sync.dma_start(out=out_d.ap()[ts(mt, P), ts(no, NCD)], in_=oe[:])
        p6ps.release()
        p6.release()
        q2p.release()
        cp.release()
    return nc


# ---------------------------------------------------------------- driver

def _get_compiled(key, S, D, H, DFF, ws1, ws2):
    if key in _CACHE:
        return _CACHE[key]
    from concourse import bacc

    nc = bacc.Bacc("TRN2", target_bir_lowering=False, debug=False, num_devices=NCORES)
    build_program(nc, S=S, D=D, H=H, DFF=DFF, ws1=ws1, ws2=ws2)
    nc.compile()
    _CACHE[key] = nc
    return nc


def make_in_maps(inputs):
    src = np.asarray(inputs["src"], dtype=np.float32)
    B, S, D = src.shape
    H = H_FULL
    DFF = inputs["ff1_w"].shape[0]
    arrays, ws1, ws2 = _prep_arrays(inputs, S, D, H, DFF)
    srcb = src + np.asarray(inputs["out_proj_b"], dtype=np.float32)[None, None, :]
    in_maps = []
    for c in range(NCORES):
        m = dict(arrays)
        m["src"] = np.ascontiguousarray(src[c])
        m["srcb"] = np.ascontiguousarray(srcb[c])
        in_maps.append(m)
    return in_maps, (S, D, H, DFF, ws1, ws2)


def kernel(**inputs):
    from concourse.bass_utils import run_bass_kernel_spmd

    in_maps, (S, D, H, DFF, ws1, ws2) = make_in_maps(inputs)
    assert np.asarray(inputs["src"]).shape[0] == NCORES
    nc = _get_compiled(("full", S, D, H, DFF, ws1, ws2), S, D, H, DFF, ws1, ws2)
    res = run_bass_kernel_spmd(nc, in_maps, core_ids=list(range(NCORES)))
    out = np.stack([res.results[c]["out"] for c in range(NCORES)], axis=0)
    return out.astype(np.float32)
